# revision 26
# baseline (speedup 1.0000x reference)
"""Local (7x7 windowed) attention Trainium2 kernel, v3 (bf16).

Problem: B=1, N=4096 (T=4, H=W=32), C=384, 8 heads x hd=48, window 7x7
zero-padded (reference semantics: padded keys score exactly 0 -> weight
exp(0), value 0).

Sharding: data-parallel over positions. 8 cores; core c owns t-slice
c//2, query rows [16*(c%2), 16*(c%2)+16) (512 queries). Each core
recomputes k/v for a 3-row halo (24 rows = 768 halo positions,
zero-padded outside the image, matching the reference's zero padding).

v3 notes (each from trace evidence):
 - all matmuls bf16: 1 cyc/row at any N; fp32 ran 1.5-4 cyc/row and let
   the PE HAM clock drop to 1.2 GHz.
 - window mask = multiplicative 0/1 bf16 mask on exp(S), split between
   DVE and GpSimd (PE additive-mask matmuls were ~10k wasted rows).
 - one big exp per head instead of 3-6 small ones: ACT costs ~390 ns
   fixed per instruction on HW.
 - q and k share one 3-bank PSUM tile so evacuation is one copy per pr.
 - noob (x-out-of-bounds exp(0) count) folded into the O->SBUF copy as
   a tensor_add, denominators gathered across all 4 pr into one [8,512]
   and inverted with one reciprocal_approx_fast (DVE reciprocal costs
   6.5 ns/col; ACT Reciprocal is blocked by bass).
 - input DMAs spread across queues (sync: xT; gpsimd: consts) - 16 DMAs
   on one queue serialized ~12 us of startup in v2.
"""

import os

import numpy as np
import ml_dtypes

import concourse.bacc as bacc
import concourse.mybir as mybir
import concourse.tile as tile
from concourse.bass_utils import run_bass_kernel_spmd

F = mybir.dt.float32
R = mybir.dt.float32r
BF = mybir.dt.bfloat16
BF_NP = ml_dtypes.bfloat16

NH = 8
HD = 48
T, HH, WW = 4, 32, 32
C = 384
NPOS = T * HH * WW
SCALE = HD ** -0.5

# per key-tile jt (4 halo key rows each): (jt, ilo, span) in owned-query
# coords. jt5's span is extended 64->128 so the packed S layout has no
# uninitialized gap (the extra (k,q) pairs are out-of-window -> masked).
SPANS = [
    (0, 0, 128),
    (1, 0, 256),
    (2, 64, 320),
    (3, 192, 320),
    (4, 320, 192),
    (5, 384, 128),
]
# packed column offsets inside the [128, 1344] S/eT layout
# (bank0: jt0,jt1,jt5 = 512; bank1: jt2,jt4 = 512; bank2: jt3 = 320)
S_OFF = {0: 0, 1: 128, 5: 384, 2: 512, 4: 832, 3: 1024}
EW = 1344

_CACHE = {}
LAST_RESULT = None


def _build_nc():
    if "nc" in _CACHE:
        return _CACHE["nc"]
    nc = bacc.Bacc("TRN2", target_bir_lowering=False)

    d_xT = nc.dram_tensor("xT", [128, 3, 768], BF, kind="ExternalInput")
    d_wqk = nc.dram_tensor("wqk", [128, 3, 8, 128], BF, kind="ExternalInput")
    d_wv = nc.dram_tensor("wv", [128, 3, 384], BF, kind="ExternalInput")
    d_wp = nc.dram_tensor("wp", [128, 4, 384], BF, kind="ExternalInput")
    d_bp = nc.dram_tensor("bp", [1, 384], BF, kind="ExternalInput")
    d_mneg = nc.dram_tensor("mneg", [128, EW], BF, kind="ExternalInput")
    d_ident = nc.dram_tensor("ident", [128, 128], BF, kind="ExternalInput")
    d_noobp = nc.dram_tensor("noobp", [128, 512], BF, kind="ExternalInput")
    d_sel4 = nc.dram_tensor("sel4", [128, 2, 4], BF, kind="ExternalInput")
    d_bsel = nc.dram_tensor("bsel", [4, 2, 128], R, kind="ExternalInput")
    d_ones1 = nc.dram_tensor("ones1", [1, 128], BF, kind="ExternalInput")
    d_vtall = nc.dram_tensor("vtall", [128, 6, 8, 16], BF, kind="ExternalInput")
    d_out = nc.dram_tensor("out", [512, 384], F, kind="ExternalOutput")

    EXP = mybir.ActivationFunctionType.Exp

    with tile.TileContext(nc) as tc:
        with tc.tile_pool(name="singles", bufs=1) as S:
            xT = S.tile([128, 3, 768], BF)
            wqk = S.tile([128, 3, 8, 128], BF)
            wv = S.tile([128, 3, 384], BF)
            wp = S.tile([128, 4, 384], BF)
            bp = S.tile([1, 384], BF)
            mneg = S.tile([128, EW], BF)
            ident = S.tile([128, 128], BF)
            noobp = S.tile([128, 512], BF)
            sel4 = S.tile([128, 2, 4], BF)
            bsel = S.tile([4, 2, 128], R)
            ones1 = S.tile([1, 128], BF)
            qkT2 = S.tile([128, 4, 1280], BF)
            vaug = S.tile([128, 6, 8, 64], BF)
            eTall = S.tile([128, 8, EW], BF)
            nhat = S.tile([128, 4, 512], BF)

            # Spread input DMAs over four queues so transfers overlap;
            # the first matmul needs only xT[k0] + wqk[k0] (both first on
            # sync). One 786KB wqk DMA measured ~7us at ~111GB/s, so wqk
            # is split per k-slice across queues.
            nc.sync.dma_start(out=xT[:, 0, :], in_=d_xT[:, 0, :])
            nc.sync.dma_start(out=wqk[:, 0, :, :], in_=d_wqk[:, 0, :, :])
            nc.sync.dma_start(out=xT[:, 1, :], in_=d_xT[:, 1, :])
            nc.sync.dma_start(out=xT[:, 2, :], in_=d_xT[:, 2, :])
            nc.scalar.dma_start(out=wqk[:, 1, :, :], in_=d_wqk[:, 1, :, :])
            nc.scalar.dma_start(out=wqk[:, 2, :, :], in_=d_wqk[:, 2, :, :])
            nc.scalar.dma_start(out=ident[:], in_=d_ident[:])
            nc.scalar.dma_start(out=wp[:], in_=d_wp[:])
            nc.scalar.dma_start(out=sel4[:], in_=d_sel4[:])
            nc.gpsimd.dma_start(out=wv[:], in_=d_wv[:])
            nc.gpsimd.dma_start(out=mneg[:], in_=d_mneg[:])
            nc.gpsimd.dma_start(out=noobp[:], in_=d_noobp[:])
            nc.gpsimd.dma_start(out=bsel[:], in_=d_bsel[:])
            nc.gpsimd.dma_start(out=ones1[:], in_=d_ones1[:])
            nc.gpsimd.dma_start(out=bp[:], in_=d_bp[:])
            # vaug's denominator-ones column (col 48) + zero pad 49:63
            nc.gpsimd.dma_start(out=vaug[:, :, :, 48:64], in_=d_vtall[:])

            # ---- P1: q (owned 512) + k (halo 768) in one PSUM tile ----
            with tc.tile_pool(name="psA", bufs=2, space="PSUM") as psA:
                for pr in range(4):
                    QK = psA.tile([128, 1536], F, tag="QK")
                    for k in range(3):
                        st, sp_ = (k == 0), (k == 2)
                        nc.tensor.matmul(QK[:, 0:512], wqk[:, k, 2 * pr, :],
                                         xT[:, k, 96:608], start=st, stop=sp_)
                        nc.tensor.matmul(QK[:, 512:1024],
                                         wqk[:, k, 2 * pr + 1, :],
                                         xT[:, k, 0:512], start=st, stop=sp_)
                        nc.tensor.matmul(QK[:, 1024:1280],
                                         wqk[:, k, 2 * pr + 1, :],
                                         xT[:, k, 512:768], start=st, stop=sp_)
                    nc.scalar.copy(qkT2[:, pr, :], QK[:, 0:1280])
                for pt in range(6):
                    V = psA.tile([128, 384], F, tag="V")
                    for k in range(3):
                        nc.tensor.matmul(V[:], xT[:, k, 128 * pt:128 * pt + 128],
                                         wv[:, k, :], start=(k == 0), stop=(k == 2))
                    nc.vector.tensor_copy(
                        vaug[:, pt, :, 0:48],
                        V[:].rearrange("p (h d) -> p h d", h=8))

            # ---- P2..P5 in ONE PSUM pool (no pool-transition barriers)
            # Bank budget (8): S 3 + O 1 + D 1 + Bc 1 + P 2x1 = 8.
            # Per head h: 6 mask matmuls (ident stationary, -300 window
            # mask) then 6 score matmuls accumulate (start=True only on
            # the first matmul touching each PSUM bank: start clears the
            # whole bank's has_written bits). exp is split at column 512
            # so the next head's bank-0 matmuls only wait on exp part 1.
            # V-matmuls of head h-1 fill the PE while exp(h) runs; both
            # heads of a pr pair-stack into one O bank (tile_position
            # auto-derived from the output base partition).
            BANK_FIRST = {0, 2, 3}
            BANK_LAST = {5, 4, 3}
            JT_B0 = [(0, 0, 128), (1, 0, 256), (5, 384, 128)]
            JT_B12 = [(2, 64, 320), (4, 320, 192), (3, 192, 320)]
            with tc.tile_pool(name="psW", bufs=1, space="PSUM") as psW, \
                 tc.tile_pool(name="sbn", bufs=2) as sbn, \
                 tc.tile_pool(name="sbo", bufs=2) as sbo:
                oTs = {}
                for pr in range(4):
                    oTs[pr] = sbo.tile([128, 512], BF, tag=f"oT{pr % 2}",
                                       name=f"oT{pr}")
                Ps = {}
                recrs = {}

                def s_mms(h):
                    pr, e = h // 2, h % 2
                    Sb = psW.tile([128, 1536], F, tag="S", name="S")
                    for jt, ilo, spn in JT_B0 + JT_B12:
                        so = S_OFF[jt]
                        nc.tensor.matmul(
                            Sb[:, so:so + spn], ident[:], mneg[:, so:so + spn],
                            start=(jt in BANK_FIRST), stop=False,
                            skip_group_check=True)
                    for jt, ilo, spn in JT_B0 + JT_B12:
                        so = S_OFF[jt]
                        nc.tensor.matmul(
                            Sb[:, so:so + spn],
                            qkT2[64 * e:64 * e + 64, pr,
                                 512 + 128 * jt:512 + 128 * (jt + 1)],
                            qkT2[64 * e:64 * e + 64, pr, ilo:ilo + spn],
                            start=False, stop=(jt in BANK_LAST),
                            skip_group_check=True)
                    nc.scalar.activation(eTall[:, h, 0:512], Sb[:, 0:512],
                                         EXP, scale=SCALE)
                    nc.scalar.activation(eTall[:, h, 512:EW], Sb[:, 512:EW],
                                         EXP, scale=SCALE)

                def v_mms(pr):
                    O = psW.tile([128, 512], F, tag="O", name="O")
                    for e in range(2):
                        h = 2 * pr + e
                        for i, (jt, ilo, spn) in enumerate(SPANS):
                            nc.tensor.matmul(
                                O[64 * e:64 * e + 64, ilo:ilo + spn],
                                vaug[:, jt, h, :],
                                eTall[:, h, S_OFF[jt]:S_OFF[jt] + spn],
                                start=(i == 0), stop=(i == 5),
                                skip_group_check=True)
                    # evacuate both heads + add noob to den rows 48/112
                    nc.vector.tensor_add(oTs[pr][:], O[:], noobp[:])

                def d_mms(g):  # g = 0: pr 0,1; g = 1: pr 2,3
                    D = psW.tile([4, 512], F, tag="D", name="D")
                    for j in range(2):
                        nc.tensor.matmul(D[:], sel4[:, j, :], oTs[2 * g + j][:],
                                         start=(j == 0), stop=(j == 1),
                                         skip_group_check=True)
                    recf = sbn.tile([4, 512], F, tag="recf", name="recf")
                    nc.vector.reciprocal_approx_fast(recf[:], D[:])
                    recr = sbn.tile([4, 512], R, tag="recr", name="recr")
                    nc.vector.tensor_copy(recr[:], recf[:])
                    recrs[g] = recr

                def bc_nhat(pr):
                    Bc = psW.tile([128, 512], F, tag="Bc", name="Bc")
                    nc.tensor.matmul(Bc[:], bsel[:, pr % 2, :],
                                     recrs[pr // 2][:], start=True, stop=True)
                    nc.vector.tensor_mul(nhat[:, pr, :], oTs[pr][:], Bc[:])

                def p5(it, pr):
                    if pr == 0:
                        Ps[it] = psW.tile([128, 512], F, tag="P", bufs=2,
                                          name=f"P{it}")
                    nc.tensor.matmul(
                        Ps[it][:, 0:384], nhat[:, pr, 128 * it:128 * (it + 1)],
                        wp[:, pr, :], start=(pr == 0), stop=False,
                        skip_group_check=True)

                def p5_out(it, eng):
                    nc.tensor.matmul(Ps[it][:, 0:384], ones1[:], bp[:],
                                     start=False, stop=True,
                                     skip_group_check=True)
                    ot = sbo.tile([128, 384], F, tag="ot", name=f"ot{it}")
                    if eng == 0:
                        nc.scalar.copy(ot[:], Ps[it][:, 0:384])
                    else:
                        nc.vector.tensor_copy(ot[:], Ps[it][:, 0:384])
                    nc.gpsimd.dma_start(out=d_out[128 * it:128 * (it + 1), :],
                                        in_=ot[:])

                s_mms(0)
                s_mms(1)
                s_mms(2)
                v_mms(0)
                s_mms(3)
                s_mms(4)
                v_mms(1)
                s_mms(5)
                d_mms(0)
                s_mms(6)
                v_mms(2)
                s_mms(7)
                v_mms(3)
                d_mms(1)
                bc_nhat(0)
                bc_nhat(1)
                p5(0, 0)
                p5(1, 0)
                p5(0, 1)
                p5(1, 1)
                bc_nhat(2)
                bc_nhat(3)
                p5(0, 2)
                p5(1, 2)
                p5(0, 3)
                p5(1, 3)
                p5_out(0, 0)
                p5_out(1, 1)
                for it in (2, 3):
                    for pr in range(4):
                        p5(it, pr)
                p5_out(2, 0)
                p5_out(3, 1)

    nc.compile()
    _CACHE["nc"] = nc
    return nc


def _host_consts(w_qkv, w_proj, b_proj):
    wqk = np.zeros((128, 3, 8, 128), np.float32)
    for k in range(3):
        rows = slice(k * 128, (k + 1) * 128)
        for pr in range(4):
            for s in range(2):  # 0 = q block, 1 = k block
                off = 384 * s
                wqk[:, k, 2 * pr + s, 0:48] = \
                    w_qkv[rows, off + 48 * (2 * pr):off + 48 * (2 * pr) + 48]
                wqk[:, k, 2 * pr + s, 64:112] = \
                    w_qkv[rows, off + 48 * (2 * pr + 1):off + 48 * (2 * pr + 1) + 48]
    wvp = np.ascontiguousarray(np.transpose(
        w_qkv[:, 768:1152].reshape(3, 128, 384), (1, 0, 2)))
    wp = np.zeros((128, 4, 384), np.float32)
    for pr in range(4):
        wp[0:48, pr, :] = w_proj[96 * pr:96 * pr + 48, :]
        wp[64:112, pr, :] = w_proj[96 * pr + 48:96 * pr + 96, :]
    bp = b_proj.reshape(1, 384)

    # additive window mask in the packed S layout: entry (k, q) of tile
    # jt is in-window iff |key_halo_row - query_halo_row| <= 3 and
    # |kx - qx| <= 3; out-of-window scores get -300 (exp -> ~1e-19).
    mneg = np.zeros((128, EW), np.float32)
    kk = np.arange(128)
    for jt, ilo, spn in SPANS:
        q = np.arange(ilo, ilo + spn)
        krow = 4 * jt + kk[:, None] // 32
        qrow = q[None, :] // 32 + 3
        kx = kk[:, None] % 32
        qx = q[None, :] % 32
        good = (np.abs(krow - qrow) <= 3) & (np.abs(kx - qx) <= 3)
        mneg[:, S_OFF[jt]:S_OFF[jt] + spn] = np.where(good, 0.0, -300.0)

    # noob folded into the O->oT copy: denominator rows 48 (e=0) and 112
    # (e=1) get the count of x-out-of-bounds keys (reference zero-pads
    # those -> exp(0) each).
    noobp = np.zeros((128, 512), np.float32)
    for qy in range(16):
        for qx in range(32):
            nb = 7.0 * (max(0, 3 - qx) + max(0, qx - 28))
            noobp[48, 32 * qy + qx] = nb
            noobp[112, 32 * qy + qx] = nb
    sel4 = np.zeros((128, 2, 4), np.float32)
    for j in range(2):
        sel4[48, j, 2 * j] = 1.0
        sel4[112, j, 2 * j + 1] = 1.0
    bsel = np.zeros((4, 2, 128), np.float32)
    for j in range(2):
        bsel[2 * j, j, 0:64] = 1.0
        bsel[2 * j + 1, j, 64:128] = 1.0
    ones1 = np.ones((1, 128), np.float32)
    identm = np.eye(128, dtype=np.float32)
    vtall = np.zeros((128, 6, 8, 16), np.float32)
    vtall[:, :, :, 0] = 1.0
    c = dict(wqk=wqk, wv=wvp, wp=wp, bp=bp, mneg=mneg, noobp=noobp,
             sel4=sel4, ones1=ones1, ident=identm, vtall=vtall)
    out = {k: np.ascontiguousarray(v.astype(BF_NP)) for k, v in c.items()}
    out["bsel"] = np.ascontiguousarray(bsel)  # fp32r stays fp32 bits
    return out


def kernel(x, w_qkv, w_proj, b_proj, H=32, W=32):
    global LAST_RESULT
    x = np.asarray(x, np.float32)
    w_qkv = np.asarray(w_qkv, np.float32)
    w_proj = np.asarray(w_proj, np.float32)
    b_proj = np.asarray(b_proj, np.float32)
    assert x.shape == (1, NPOS, C) and int(H) == 32 and int(W) == 32

    nc = _build_nc()
    consts = _host_consts(w_qkv, w_proj, b_proj)

    x4 = x[0].reshape(T, HH, WW, C)
    in_maps = []
    for c in range(8):
        t, ry0 = c // 2, 16 * (c % 2)
        xh = np.zeros((24, WW, C), np.float32)
        lo, hi = ry0 - 3, ry0 + 21
        slo, shi = max(lo, 0), min(hi, HH)
        xh[slo - lo:shi - lo] = x4[t, slo:shi]
        xT = np.ascontiguousarray(
            xh.reshape(768, C).T.reshape(3, 128, 768).transpose(1, 0, 2)
        ).astype(BF_NP)
        in_maps.append({"xT": xT, **consts})

    trace = bool(int(os.environ.get("TRACE", "0")))
    res = run_bass_kernel_spmd(nc, in_maps, core_ids=list(range(8)),
                               trace=trace)
    LAST_RESULT = res
    out = np.concatenate([res.results[c]["out"] for c in range(8)], axis=0)
    return out.reshape(1, NPOS, C)


# revision 29
# speedup vs baseline: 1.0374x; 1.0374x over previous
"""Local (7x7 windowed) attention Trainium2 kernel, v3 (bf16).

Problem: B=1, N=4096 (T=4, H=W=32), C=384, 8 heads x hd=48, window 7x7
zero-padded (reference semantics: padded keys score exactly 0 -> weight
exp(0), value 0).

Sharding: data-parallel over positions. 8 cores; core c owns t-slice
c//2, query rows [16*(c%2), 16*(c%2)+16) (512 queries). Each core
recomputes k/v for a 3-row halo (24 rows = 768 halo positions,
zero-padded outside the image, matching the reference's zero padding).

v3 notes (each from trace evidence):
 - all matmuls bf16: 1 cyc/row at any N; fp32 ran 1.5-4 cyc/row and let
   the PE HAM clock drop to 1.2 GHz.
 - window mask = multiplicative 0/1 bf16 mask on exp(S), split between
   DVE and GpSimd (PE additive-mask matmuls were ~10k wasted rows).
 - one big exp per head instead of 3-6 small ones: ACT costs ~390 ns
   fixed per instruction on HW.
 - q and k share one 3-bank PSUM tile so evacuation is one copy per pr.
 - noob (x-out-of-bounds exp(0) count) folded into the O->SBUF copy as
   a tensor_add, denominators gathered across all 4 pr into one [8,512]
   and inverted with one reciprocal_approx_fast (DVE reciprocal costs
   6.5 ns/col; ACT Reciprocal is blocked by bass).
 - input DMAs spread across queues (sync: xT; gpsimd: consts) - 16 DMAs
   on one queue serialized ~12 us of startup in v2.
"""

import os

import numpy as np
import ml_dtypes

import concourse.bacc as bacc
import concourse.mybir as mybir
import concourse.tile as tile
from concourse.bass_utils import run_bass_kernel_spmd

F = mybir.dt.float32
R = mybir.dt.float32r
BF = mybir.dt.bfloat16
BF_NP = ml_dtypes.bfloat16

NH = 8
HD = 48
T, HH, WW = 4, 32, 32
C = 384
NPOS = T * HH * WW
SCALE = HD ** -0.5

# per key-tile jt (4 halo key rows each): (jt, ilo, span) in owned-query
# coords. jt5's span is extended 64->128 so the packed S layout has no
# uninitialized gap (the extra (k,q) pairs are out-of-window -> masked).
SPANS = [
    (0, 0, 128),
    (1, 0, 256),
    (2, 64, 320),
    (3, 192, 320),
    (4, 320, 192),
    (5, 384, 128),
]
# packed column offsets inside the [128, 1344] S/eT layout
# (bank0: jt0,jt1,jt5 = 512; bank1: jt2,jt4 = 512; bank2: jt3 = 320)
S_OFF = {0: 0, 1: 128, 5: 384, 2: 512, 4: 832, 3: 1024}
EW = 1344

_CACHE = {}
LAST_RESULT = None


def _build_nc():
    if "nc" in _CACHE:
        return _CACHE["nc"]
    nc = bacc.Bacc("TRN2", target_bir_lowering=False)

    d_xT = nc.dram_tensor("xT", [128, 3, 768], BF, kind="ExternalInput")
    d_wqk = nc.dram_tensor("wqk", [128, 3, 8, 128], BF, kind="ExternalInput")
    d_wv = nc.dram_tensor("wv", [128, 3, 384], BF, kind="ExternalInput")
    d_wp = nc.dram_tensor("wp", [128, 4, 384], BF, kind="ExternalInput")
    d_bp = nc.dram_tensor("bp", [1, 384], BF, kind="ExternalInput")
    d_mneg = nc.dram_tensor("mneg", [128, EW], BF, kind="ExternalInput")
    d_ident = nc.dram_tensor("ident", [128, 128], BF, kind="ExternalInput")
    d_noobp = nc.dram_tensor("noobp", [128, 512], BF, kind="ExternalInput")
    d_sel4 = nc.dram_tensor("sel4", [128, 2, 4], BF, kind="ExternalInput")
    d_bsel = nc.dram_tensor("bsel", [4, 2, 128], R, kind="ExternalInput")
    d_ones1 = nc.dram_tensor("ones1", [1, 128], BF, kind="ExternalInput")
    d_vtall = nc.dram_tensor("vtall", [128, 6, 8, 16], BF, kind="ExternalInput")
    d_out = nc.dram_tensor("out", [512, 384], F, kind="ExternalOutput")

    EXP = mybir.ActivationFunctionType.Exp

    with tile.TileContext(nc) as tc:
        with tc.tile_pool(name="singles", bufs=1) as S:
            xT = S.tile([128, 3, 768], BF)
            wqk = S.tile([128, 3, 8, 128], BF)
            wv = S.tile([128, 3, 384], BF)
            wp = S.tile([128, 4, 384], BF)
            bp = S.tile([1, 384], BF)
            mneg = S.tile([128, EW], BF)
            ident = S.tile([128, 128], BF)
            noobp = S.tile([128, 512], BF)
            sel4 = S.tile([128, 2, 4], BF)
            bsel = S.tile([4, 2, 128], R)
            ones1 = S.tile([1, 128], BF)
            qkT2 = S.tile([128, 4, 1280], BF)
            vaug = S.tile([128, 6, 8, 64], BF)
            eTall = S.tile([128, 8, EW], BF)
            nhat = S.tile([128, 4, 512], BF)

            # Spread input DMAs over four queues so transfers overlap;
            # the first matmul needs only xT[k0] + wqk[k0] (both first on
            # sync). One 786KB wqk DMA measured ~7us at ~111GB/s, so wqk
            # is split per k-slice across queues.
            nc.sync.dma_start(out=wqk[:, 0, 0:2, :], in_=d_wqk[:, 0, 0:2, :])
            nc.sync.dma_start(out=wqk[:, 0, 2:8, :], in_=d_wqk[:, 0, 2:8, :])
            nc.sync.dma_start(out=wqk[:, 1, :, :], in_=d_wqk[:, 1, :, :])
            nc.sync.dma_start(out=wqk[:, 2, :, :], in_=d_wqk[:, 2, :, :])
            nc.scalar.dma_start(out=xT[:, 0, :], in_=d_xT[:, 0, :])
            nc.scalar.dma_start(out=xT[:, 1, :], in_=d_xT[:, 1, :])
            nc.scalar.dma_start(out=xT[:, 2, :], in_=d_xT[:, 2, :])
            nc.scalar.dma_start(out=ident[:], in_=d_ident[:])
            nc.scalar.dma_start(out=wp[:], in_=d_wp[:])
            nc.scalar.dma_start(out=sel4[:], in_=d_sel4[:])
            nc.gpsimd.dma_start(out=wv[:], in_=d_wv[:])
            nc.gpsimd.dma_start(out=mneg[:], in_=d_mneg[:])
            nc.gpsimd.dma_start(out=noobp[:], in_=d_noobp[:])
            nc.gpsimd.dma_start(out=bsel[:], in_=d_bsel[:])
            nc.gpsimd.dma_start(out=ones1[:], in_=d_ones1[:])
            nc.gpsimd.dma_start(out=bp[:], in_=d_bp[:])
            # vaug's denominator-ones column (col 48) + zero pad 49:63
            nc.gpsimd.dma_start(out=vaug[:, :, :, 48:64], in_=d_vtall[:])

            # ---- P1: q (owned 512) + k (halo 768) in one PSUM tile ----
            with tc.tile_pool(name="psA", bufs=2, space="PSUM") as psA:
                for pr in range(4):
                    QK = psA.tile([128, 1536], F, tag="QK")
                    for k in range(3):
                        st, sp_ = (k == 0), (k == 2)
                        nc.tensor.matmul(QK[:, 0:512], wqk[:, k, 2 * pr, :],
                                         xT[:, k, 96:608], start=st, stop=sp_)
                        nc.tensor.matmul(QK[:, 512:1024],
                                         wqk[:, k, 2 * pr + 1, :],
                                         xT[:, k, 0:512], start=st, stop=sp_)
                        nc.tensor.matmul(QK[:, 1024:1280],
                                         wqk[:, k, 2 * pr + 1, :],
                                         xT[:, k, 512:768], start=st, stop=sp_)
                    nc.vector.tensor_copy(qkT2[:, pr, :], QK[:, 0:1280])
                for pt in range(6):
                    V = psA.tile([128, 384], F, tag="V")
                    for k in range(3):
                        nc.tensor.matmul(V[:], xT[:, k, 128 * pt:128 * pt + 128],
                                         wv[:, k, :], start=(k == 0), stop=(k == 2))
                    nc.scalar.copy(
                        vaug[:, pt, :, 0:48],
                        V[:].rearrange("p (h d) -> p h d", h=8))

            # ---- P2..P5 in ONE PSUM pool (no pool-transition barriers)
            # Bank budget (8): S 3 + O 1 + D 1 + Bc 1 + P 2x1 = 8.
            # Per head h: 6 mask matmuls (ident stationary, -300 window
            # mask) then 6 score matmuls accumulate (start=True only on
            # the first matmul touching each PSUM bank: start clears the
            # whole bank's has_written bits). exp is split at column 512
            # so the next head's bank-0 matmuls only wait on exp part 1.
            # V-matmuls of head h-1 fill the PE while exp(h) runs; both
            # heads of a pr pair-stack into one O bank (tile_position
            # auto-derived from the output base partition).
            BANK_FIRST = {0, 2, 3}
            BANK_LAST = {5, 4, 3}
            JT_B0 = [(0, 0, 128), (1, 0, 256), (5, 384, 128)]
            JT_B12 = [(2, 64, 320), (4, 320, 192), (3, 192, 320)]
            with tc.tile_pool(name="psW", bufs=1, space="PSUM") as psW, \
                 tc.tile_pool(name="sbn", bufs=2) as sbn, \
                 tc.tile_pool(name="sbo", bufs=2) as sbo:
                oTs = {}
                for pr in range(4):
                    oTs[pr] = sbo.tile([128, 512], BF, tag=f"oT{pr % 2}",
                                       name=f"oT{pr}")
                Ps = {}
                recrs = {}

                def s_mms(h):
                    pr, e = h // 2, h % 2
                    Sb = psW.tile([128, 1536], F, tag="S", name="S")
                    for jt, ilo, spn in JT_B0 + JT_B12:
                        so = S_OFF[jt]
                        nc.tensor.matmul(
                            Sb[:, so:so + spn], ident[:], mneg[:, so:so + spn],
                            start=(jt in BANK_FIRST), stop=False,
                            skip_group_check=True)
                    for jt, ilo, spn in JT_B0 + JT_B12:
                        so = S_OFF[jt]
                        nc.tensor.matmul(
                            Sb[:, so:so + spn],
                            qkT2[64 * e:64 * e + 64, pr,
                                 512 + 128 * jt:512 + 128 * (jt + 1)],
                            qkT2[64 * e:64 * e + 64, pr, ilo:ilo + spn],
                            start=False, stop=(jt in BANK_LAST),
                            skip_group_check=True)
                    nc.scalar.activation(eTall[:, h, 0:512], Sb[:, 0:512],
                                         EXP, scale=SCALE)
                    nc.scalar.activation(eTall[:, h, 512:EW], Sb[:, 512:EW],
                                         EXP, scale=SCALE)

                def v_mms(pr):
                    O = psW.tile([128, 512], F, tag="O", name="O")
                    for e in range(2):
                        h = 2 * pr + e
                        for i, (jt, ilo, spn) in enumerate(SPANS):
                            nc.tensor.matmul(
                                O[64 * e:64 * e + 64, ilo:ilo + spn],
                                vaug[:, jt, h, :],
                                eTall[:, h, S_OFF[jt]:S_OFF[jt] + spn],
                                start=(i == 0), stop=(i == 5),
                                skip_group_check=True)
                    # evacuate both heads + add noob to den rows 48/112
                    nc.vector.tensor_add(oTs[pr][:], O[:], noobp[:])

                def d_mms(g):  # g = 0: pr 0,1; g = 1: pr 2,3
                    D = psW.tile([4, 512], F, tag="D", name="D")
                    for j in range(2):
                        nc.tensor.matmul(D[:], sel4[:, j, :], oTs[2 * g + j][:],
                                         start=(j == 0), stop=(j == 1),
                                         skip_group_check=True)
                    recf = sbn.tile([4, 512], F, tag="recf", name="recf")
                    nc.vector.reciprocal_approx_fast(recf[:], D[:])
                    recr = sbn.tile([4, 512], R, tag="recr", name="recr")
                    nc.vector.tensor_copy(recr[:], recf[:])
                    recrs[g] = recr

                def bc_nhat(pr):
                    Bc = psW.tile([128, 512], F, tag="Bc", name="Bc")
                    nc.tensor.matmul(Bc[:], bsel[:, pr % 2, :],
                                     recrs[pr // 2][:], start=True, stop=True)
                    nc.vector.tensor_mul(nhat[:, pr, :], oTs[pr][:], Bc[:])

                def p5(it, pr):
                    if pr == 0:
                        Ps[it] = psW.tile([128, 512], F, tag="P", bufs=2,
                                          name=f"P{it}")
                    nc.tensor.matmul(
                        Ps[it][:, 0:384], nhat[:, pr, 128 * it:128 * (it + 1)],
                        wp[:, pr, :], start=(pr == 0), stop=False,
                        skip_group_check=True)

                def p5_out(it, eng):
                    nc.tensor.matmul(Ps[it][:, 0:384], ones1[:], bp[:],
                                     start=False, stop=True,
                                     skip_group_check=True)
                    ot = sbo.tile([128, 384], F, tag="ot", name=f"ot{it}")
                    if eng == 0:
                        nc.scalar.copy(ot[:], Ps[it][:, 0:384])
                    else:
                        nc.vector.tensor_copy(ot[:], Ps[it][:, 0:384])
                    nc.gpsimd.dma_start(out=d_out[128 * it:128 * (it + 1), :],
                                        in_=ot[:])

                s_mms(0)
                s_mms(1)
                s_mms(2)
                v_mms(0)
                s_mms(3)
                s_mms(4)
                v_mms(1)
                s_mms(5)
                d_mms(0)
                s_mms(6)
                v_mms(2)
                s_mms(7)
                v_mms(3)
                d_mms(1)
                bc_nhat(0)
                bc_nhat(1)
                p5(0, 0)
                p5(1, 0)
                p5(0, 1)
                p5(1, 1)
                bc_nhat(2)
                bc_nhat(3)
                p5(0, 2)
                p5(1, 2)
                p5(0, 3)
                p5(1, 3)
                p5_out(0, 0)
                p5_out(1, 1)
                for it in (2, 3):
                    for pr in range(4):
                        p5(it, pr)
                p5_out(2, 0)
                p5_out(3, 1)

    nc.compile()
    _CACHE["nc"] = nc
    return nc


def _host_consts(w_qkv, w_proj, b_proj):
    wqk = np.zeros((128, 3, 8, 128), np.float32)
    for k in range(3):
        rows = slice(k * 128, (k + 1) * 128)
        for pr in range(4):
            for s in range(2):  # 0 = q block, 1 = k block
                off = 384 * s
                wqk[:, k, 2 * pr + s, 0:48] = \
                    w_qkv[rows, off + 48 * (2 * pr):off + 48 * (2 * pr) + 48]
                wqk[:, k, 2 * pr + s, 64:112] = \
                    w_qkv[rows, off + 48 * (2 * pr + 1):off + 48 * (2 * pr + 1) + 48]
    wvp = np.ascontiguousarray(np.transpose(
        w_qkv[:, 768:1152].reshape(3, 128, 384), (1, 0, 2)))
    wp = np.zeros((128, 4, 384), np.float32)
    for pr in range(4):
        wp[0:48, pr, :] = w_proj[96 * pr:96 * pr + 48, :]
        wp[64:112, pr, :] = w_proj[96 * pr + 48:96 * pr + 96, :]
    bp = b_proj.reshape(1, 384)

    # additive window mask in the packed S layout: entry (k, q) of tile
    # jt is in-window iff |key_halo_row - query_halo_row| <= 3 and
    # |kx - qx| <= 3; out-of-window scores get -300 (exp -> ~1e-19).
    mneg = np.zeros((128, EW), np.float32)
    kk = np.arange(128)
    for jt, ilo, spn in SPANS:
        q = np.arange(ilo, ilo + spn)
        krow = 4 * jt + kk[:, None] // 32
        qrow = q[None, :] // 32 + 3
        kx = kk[:, None] % 32
        qx = q[None, :] % 32
        good = (np.abs(krow - qrow) <= 3) & (np.abs(kx - qx) <= 3)
        mneg[:, S_OFF[jt]:S_OFF[jt] + spn] = np.where(good, 0.0, -300.0)

    # noob folded into the O->oT copy: denominator rows 48 (e=0) and 112
    # (e=1) get the count of x-out-of-bounds keys (reference zero-pads
    # those -> exp(0) each).
    noobp = np.zeros((128, 512), np.float32)
    for qy in range(16):
        for qx in range(32):
            nb = 7.0 * (max(0, 3 - qx) + max(0, qx - 28))
            noobp[48, 32 * qy + qx] = nb
            noobp[112, 32 * qy + qx] = nb
    sel4 = np.zeros((128, 2, 4), np.float32)
    for j in range(2):
        sel4[48, j, 2 * j] = 1.0
        sel4[112, j, 2 * j + 1] = 1.0
    bsel = np.zeros((4, 2, 128), np.float32)
    for j in range(2):
        bsel[2 * j, j, 0:64] = 1.0
        bsel[2 * j + 1, j, 64:128] = 1.0
    ones1 = np.ones((1, 128), np.float32)
    identm = np.eye(128, dtype=np.float32)
    vtall = np.zeros((128, 6, 8, 16), np.float32)
    vtall[:, :, :, 0] = 1.0
    c = dict(wqk=wqk, wv=wvp, wp=wp, bp=bp, mneg=mneg, noobp=noobp,
             sel4=sel4, ones1=ones1, ident=identm, vtall=vtall)
    out = {k: np.ascontiguousarray(v.astype(BF_NP)) for k, v in c.items()}
    out["bsel"] = np.ascontiguousarray(bsel)  # fp32r stays fp32 bits
    return out


def kernel(x, w_qkv, w_proj, b_proj, H=32, W=32):
    global LAST_RESULT
    x = np.asarray(x, np.float32)
    w_qkv = np.asarray(w_qkv, np.float32)
    w_proj = np.asarray(w_proj, np.float32)
    b_proj = np.asarray(b_proj, np.float32)
    assert x.shape == (1, NPOS, C) and int(H) == 32 and int(W) == 32

    nc = _build_nc()
    consts = _host_consts(w_qkv, w_proj, b_proj)

    x4 = x[0].reshape(T, HH, WW, C)
    in_maps = []
    for c in range(8):
        t, ry0 = c // 2, 16 * (c % 2)
        xh = np.zeros((24, WW, C), np.float32)
        lo, hi = ry0 - 3, ry0 + 21
        slo, shi = max(lo, 0), min(hi, HH)
        xh[slo - lo:shi - lo] = x4[t, slo:shi]
        xT = np.ascontiguousarray(
            xh.reshape(768, C).T.reshape(3, 128, 768).transpose(1, 0, 2)
        ).astype(BF_NP)
        in_maps.append({"xT": xT, **consts})

    trace = bool(int(os.environ.get("TRACE", "0")))
    res = run_bass_kernel_spmd(nc, in_maps, core_ids=list(range(8)),
                               trace=trace)
    LAST_RESULT = res
    out = np.concatenate([res.results[c]["out"] for c in range(8)], axis=0)
    return out.reshape(1, NPOS, C)


# revision 36
# speedup vs baseline: 1.0871x; 1.0479x over previous
"""Local (7x7 windowed) attention Trainium2 kernel, v3 (bf16).

Problem: B=1, N=4096 (T=4, H=W=32), C=384, 8 heads x hd=48, window 7x7
zero-padded (reference semantics: padded keys score exactly 0 -> weight
exp(0), value 0).

Sharding: data-parallel over positions. 8 cores; core c owns t-slice
c//2, query rows [16*(c%2), 16*(c%2)+16) (512 queries). Each core
recomputes k/v for a 3-row halo (24 rows = 768 halo positions,
zero-padded outside the image, matching the reference's zero padding).

v3 notes (each from trace evidence):
 - all matmuls bf16: 1 cyc/row at any N; fp32 ran 1.5-4 cyc/row and let
   the PE HAM clock drop to 1.2 GHz.
 - window mask = multiplicative 0/1 bf16 mask on exp(S), split between
   DVE and GpSimd (PE additive-mask matmuls were ~10k wasted rows).
 - one big exp per head instead of 3-6 small ones: ACT costs ~390 ns
   fixed per instruction on HW.
 - q and k share one 3-bank PSUM tile so evacuation is one copy per pr.
 - noob (x-out-of-bounds exp(0) count) folded into the O->SBUF copy as
   a tensor_add, denominators gathered across all 4 pr into one [8,512]
   and inverted with one reciprocal_approx_fast (DVE reciprocal costs
   6.5 ns/col; ACT Reciprocal is blocked by bass).
 - input DMAs spread across queues (sync: xT; gpsimd: consts) - 16 DMAs
   on one queue serialized ~12 us of startup in v2.
"""

import os

import numpy as np
import ml_dtypes

import concourse.bacc as bacc
import concourse.mybir as mybir
import concourse.tile as tile
from concourse.bass_utils import run_bass_kernel_spmd

F = mybir.dt.float32
R = mybir.dt.float32r
BF = mybir.dt.bfloat16
BF_NP = ml_dtypes.bfloat16

NH = 8
HD = 48
T, HH, WW = 4, 32, 32
C = 384
NPOS = T * HH * WW
SCALE = HD ** -0.5

# per key-tile jt (4 halo key rows each): (jt, ilo, span) in owned-query
# coords. jt5's span is extended 64->128 so the packed S layout has no
# uninitialized gap (the extra (k,q) pairs are out-of-window -> masked).
SPANS = [
    (0, 0, 128),
    (1, 0, 256),
    (2, 64, 320),
    (3, 192, 320),
    (4, 320, 192),
    (5, 384, 128),
]
# packed column offsets inside the [128, 1344] S/eT layout
# (bank0: jt0,jt1,jt5 = 512; bank1: jt2,jt4 = 512; bank2: jt3 = 320)
S_OFF = {0: 0, 1: 128, 5: 384, 2: 512, 4: 832, 3: 1024}
EW = 1344

_CACHE = {}
LAST_RESULT = None


def _build_nc():
    if "nc" in _CACHE:
        return _CACHE["nc"]
    nc = bacc.Bacc("TRN2", target_bir_lowering=False)

    d_xT = nc.dram_tensor("xT", [128, 3, 768], BF, kind="ExternalInput")
    d_wqk = nc.dram_tensor("wqk", [128, 3, 8, 128], BF, kind="ExternalInput")
    d_wv = nc.dram_tensor("wv", [128, 3, 384], BF, kind="ExternalInput")
    d_wp = nc.dram_tensor("wp", [128, 4, 384], BF, kind="ExternalInput")
    d_bp = nc.dram_tensor("bp", [1, 384], BF, kind="ExternalInput")
    d_mneg = nc.dram_tensor("mneg", [128, EW], BF, kind="ExternalInput")
    d_ident = nc.dram_tensor("ident", [128, 128], BF, kind="ExternalInput")
    d_noobp = nc.dram_tensor("noobp", [128, 512], BF, kind="ExternalInput")
    d_sel4 = nc.dram_tensor("sel4", [128, 2, 4], BF, kind="ExternalInput")
    d_bsel = nc.dram_tensor("bsel", [4, 2, 128], BF, kind="ExternalInput")
    d_ones1 = nc.dram_tensor("ones1", [1, 128], BF, kind="ExternalInput")
    d_vtall = nc.dram_tensor("vtall", [128, 6, 8, 16], BF, kind="ExternalInput")
    d_out = nc.dram_tensor("out", [512, 384], F, kind="ExternalOutput")

    EXP = mybir.ActivationFunctionType.Exp

    with tile.TileContext(nc) as tc:
        with tc.tile_pool(name="singles", bufs=1) as S:
            xT = S.tile([128, 3, 768], BF)
            wqk = S.tile([128, 3, 8, 128], BF)
            wv = S.tile([128, 3, 384], BF)
            wp = S.tile([128, 4, 384], BF)
            bp = S.tile([1, 384], BF)
            mneg = S.tile([128, EW], BF)
            ident = S.tile([128, 128], BF)
            noobp = S.tile([128, 512], BF)
            sel4 = S.tile([128, 2, 4], BF)
            bsel = S.tile([4, 2, 128], BF)
            ones1 = S.tile([1, 128], BF)
            qkT2 = S.tile([128, 4, 1280], BF)
            vaug = S.tile([128, 6, 8, 64], BF)
            eTall = S.tile([128, 8, EW], BF)
            nhat = S.tile([128, 4, 512], BF)

            # Spread input DMAs over four queues so transfers overlap;
            # the first matmul needs only xT[k0] + wqk[k0] (both first on
            # sync). One 786KB wqk DMA measured ~7us at ~111GB/s, so wqk
            # is split per k-slice across queues.
            nc.sync.dma_start(out=wqk[:, 0, 0:2, :], in_=d_wqk[:, 0, 0:2, :])
            nc.sync.dma_start(out=wqk[:, 0, 2:8, :], in_=d_wqk[:, 0, 2:8, :])
            nc.sync.dma_start(out=wqk[:, 2, :, :], in_=d_wqk[:, 2, :, :])
            nc.scalar.dma_start(out=xT[:, 0, :], in_=d_xT[:, 0, :])
            nc.scalar.dma_start(out=xT[:, 1, :], in_=d_xT[:, 1, :])
            nc.scalar.dma_start(out=xT[:, 2, :], in_=d_xT[:, 2, :])
            nc.scalar.dma_start(out=ident[:], in_=d_ident[:])
            nc.scalar.dma_start(out=wp[:], in_=d_wp[:])
            nc.scalar.dma_start(out=sel4[:], in_=d_sel4[:])
            nc.gpsimd.dma_start(out=wqk[:, 1, :, :], in_=d_wqk[:, 1, :, :])
            nc.gpsimd.dma_start(out=wv[:], in_=d_wv[:])
            nc.gpsimd.dma_start(out=mneg[:], in_=d_mneg[:])
            nc.gpsimd.dma_start(out=noobp[:], in_=d_noobp[:])
            nc.gpsimd.dma_start(out=bsel[:], in_=d_bsel[:])
            nc.gpsimd.dma_start(out=ones1[:], in_=d_ones1[:])
            nc.gpsimd.dma_start(out=bp[:], in_=d_bp[:])
            # vaug's denominator-ones column (col 48) + zero pad 49:63
            nc.gpsimd.dma_start(out=vaug[:, :, :, 48:64], in_=d_vtall[:])

            # ---- P1: q (owned 512) + k (halo 768) in one PSUM tile ----
            with tc.tile_pool(name="psA", bufs=2, space="PSUM") as psA:
                for pr in range(4):
                    QK = psA.tile([128, 1536], F, tag="QK")
                    for k in range(3):
                        st, sp_ = (k == 0), (k == 2)
                        nc.tensor.matmul(QK[:, 0:512], wqk[:, k, 2 * pr, :],
                                         xT[:, k, 96:608], start=st, stop=sp_)
                        nc.tensor.matmul(QK[:, 512:1024],
                                         wqk[:, k, 2 * pr + 1, :],
                                         xT[:, k, 0:512], start=st, stop=sp_)
                        nc.tensor.matmul(QK[:, 1024:1280],
                                         wqk[:, k, 2 * pr + 1, :],
                                         xT[:, k, 512:768], start=st, stop=sp_)
                    # pr 0/1 on ACT (idle until exps start), 2/3 on DVE
                    # so neither engine's queue delays the first exps
                    if pr < 2:
                        nc.scalar.copy(qkT2[:, pr, :], QK[:, 0:1280])
                    else:
                        nc.vector.tensor_copy(qkT2[:, pr, :], QK[:, 0:1280])
                for pt in range(6):
                    V = psA.tile([128, 384], F, tag="V")
                    for k in range(3):
                        nc.tensor.matmul(V[:], xT[:, k, 128 * pt:128 * pt + 128],
                                         wv[:, k, :], start=(k == 0), stop=(k == 2))
                    nc.vector.tensor_copy(
                        vaug[:, pt, :, 0:48],
                        V[:].rearrange("p (h d) -> p h d", h=8))

            # ---- P2..P5 in ONE PSUM pool (no pool-transition barriers)
            # Bank budget (8): S 3 + O 1 + D 1 + Bc 1 + P 2x1 = 8.
            # Per head h: 6 mask matmuls (ident stationary, -300 window
            # mask) then 6 score matmuls accumulate (start=True only on
            # the first matmul touching each PSUM bank: start clears the
            # whole bank's has_written bits). exp is split at column 512
            # so the next head's bank-0 matmuls only wait on exp part 1.
            # V-matmuls of head h-1 fill the PE while exp(h) runs; both
            # heads of a pr pair-stack into one O bank (tile_position
            # auto-derived from the output base partition).
            BANK_FIRST = {0, 2, 3}
            BANK_LAST = {5, 4, 3}
            JT_B0 = [(0, 0, 128), (1, 0, 256), (5, 384, 128)]
            JT_B12 = [(2, 64, 320), (4, 320, 192), (3, 192, 320)]
            with tc.tile_pool(name="psW", bufs=1, space="PSUM") as psW, \
                 tc.tile_pool(name="sbn", bufs=2) as sbn, \
                 tc.tile_pool(name="sbo", bufs=2) as sbo:
                oTs = {}
                for pr in range(4):
                    oTs[pr] = sbo.tile([128, 512], BF, tag=f"oT{pr % 2}",
                                       name=f"oT{pr}")
                Ps = {}
                recrs = {}

                def s_mms(h):
                    pr, e = h // 2, h % 2
                    Sb = psW.tile([128, 1536], F, tag="S", name="S")
                    for jt, ilo, spn in JT_B0 + JT_B12:
                        so = S_OFF[jt]
                        nc.tensor.matmul(
                            Sb[:, so:so + spn], ident[:], mneg[:, so:so + spn],
                            start=(jt in BANK_FIRST), stop=False,
                            skip_group_check=True)
                    for jt, ilo, spn in JT_B0 + JT_B12:
                        so = S_OFF[jt]
                        nc.tensor.matmul(
                            Sb[:, so:so + spn],
                            qkT2[64 * e:64 * e + 64, pr,
                                 512 + 128 * jt:512 + 128 * (jt + 1)],
                            qkT2[64 * e:64 * e + 64, pr, ilo:ilo + spn],
                            start=False, stop=(jt in BANK_LAST),
                            skip_group_check=True)
                    nc.scalar.activation(eTall[:, h, 0:512], Sb[:, 0:512],
                                         EXP, scale=SCALE)
                    nc.scalar.activation(eTall[:, h, 512:EW], Sb[:, 512:EW],
                                         EXP, scale=SCALE)

                def v_mms(pr):
                    O = psW.tile([128, 512], F, tag="O", name="O")
                    for e in range(2):
                        h = 2 * pr + e
                        for i, (jt, ilo, spn) in enumerate(SPANS):
                            nc.tensor.matmul(
                                O[64 * e:64 * e + 64, ilo:ilo + spn],
                                vaug[:, jt, h, :],
                                eTall[:, h, S_OFF[jt]:S_OFF[jt] + spn],
                                start=(i == 0), stop=(i == 5),
                                skip_group_check=True)
                    # evacuate both heads + add noob to den rows 48/112
                    nc.vector.tensor_add(oTs[pr][:], O[:], noobp[:])

                def d_mms(g):  # g = 0: pr 0,1; g = 1: pr 2,3
                    D = psW.tile([4, 512], F, tag="D", name="D")
                    for j in range(2):
                        nc.tensor.matmul(D[:], sel4[:, j, :], oTs[2 * g + j][:],
                                         start=(j == 0), stop=(j == 1),
                                         skip_group_check=True)
                    recf = sbn.tile([4, 512], F, tag="recf", name="recf")
                    nc.vector.reciprocal_approx_fast(recf[:], D[:])
                    recr = sbn.tile([4, 512], BF, tag="recr", name="recr")
                    nc.vector.tensor_copy(recr[:], recf[:])
                    recrs[g] = recr

                def bc_nhat(pr):
                    Bc = psW.tile([128, 512], F, tag="Bc", name="Bc")
                    nc.tensor.matmul(Bc[:], bsel[:, pr % 2, :],
                                     recrs[pr // 2][:], start=True, stop=True)
                    nc.vector.tensor_mul(nhat[:, pr, :], oTs[pr][:], Bc[:])

                def p5(it, pr):
                    if pr == 0:
                        Ps[it] = psW.tile([128, 512], F, tag="P", bufs=2,
                                          name=f"P{it}")
                    nc.tensor.matmul(
                        Ps[it][:, 0:384], nhat[:, pr, 128 * it:128 * (it + 1)],
                        wp[:, pr, :], start=(pr == 0), stop=False,
                        skip_group_check=True)

                def p5_out(it, eng):
                    nc.tensor.matmul(Ps[it][:, 0:384], ones1[:], bp[:],
                                     start=False, stop=True,
                                     skip_group_check=True)
                    ot = sbo.tile([128, 384], F, tag="ot", name=f"ot{it}")
                    if eng == 0:
                        nc.scalar.copy(ot[:], Ps[it][:, 0:384])
                    else:
                        nc.vector.tensor_copy(ot[:], Ps[it][:, 0:384])
                    nc.gpsimd.dma_start(out=d_out[128 * it:128 * (it + 1), :],
                                        in_=ot[:])

                s_mms(0)
                s_mms(1)
                s_mms(2)
                v_mms(0)
                s_mms(3)
                s_mms(4)
                v_mms(1)
                s_mms(5)
                d_mms(0)
                s_mms(6)
                v_mms(2)
                s_mms(7)
                v_mms(3)
                d_mms(1)
                bc_nhat(0)
                bc_nhat(1)
                p5(0, 0)
                p5(1, 0)
                p5(0, 1)
                p5(1, 1)
                bc_nhat(2)
                bc_nhat(3)
                p5(0, 2)
                p5(1, 2)
                p5(0, 3)
                p5(1, 3)
                p5_out(0, 0)
                p5_out(1, 1)
                for it in (2, 3):
                    for pr in range(4):
                        p5(it, pr)
                p5_out(2, 0)
                p5_out(3, 1)

    nc.compile()
    _CACHE["nc"] = nc
    return nc


def _host_consts(w_qkv, w_proj, b_proj):
    wqk = np.zeros((128, 3, 8, 128), np.float32)
    for k in range(3):
        rows = slice(k * 128, (k + 1) * 128)
        for pr in range(4):
            for s in range(2):  # 0 = q block, 1 = k block
                off = 384 * s
                wqk[:, k, 2 * pr + s, 0:48] = \
                    w_qkv[rows, off + 48 * (2 * pr):off + 48 * (2 * pr) + 48]
                wqk[:, k, 2 * pr + s, 64:112] = \
                    w_qkv[rows, off + 48 * (2 * pr + 1):off + 48 * (2 * pr + 1) + 48]
    wvp = np.ascontiguousarray(np.transpose(
        w_qkv[:, 768:1152].reshape(3, 128, 384), (1, 0, 2)))
    wp = np.zeros((128, 4, 384), np.float32)
    for pr in range(4):
        wp[0:48, pr, :] = w_proj[96 * pr:96 * pr + 48, :]
        wp[64:112, pr, :] = w_proj[96 * pr + 48:96 * pr + 96, :]
    bp = b_proj.reshape(1, 384)

    # additive window mask in the packed S layout: entry (k, q) of tile
    # jt is in-window iff |key_halo_row - query_halo_row| <= 3 and
    # |kx - qx| <= 3; out-of-window scores get -300 (exp -> ~1e-19).
    mneg = np.zeros((128, EW), np.float32)
    kk = np.arange(128)
    for jt, ilo, spn in SPANS:
        q = np.arange(ilo, ilo + spn)
        krow = 4 * jt + kk[:, None] // 32
        qrow = q[None, :] // 32 + 3
        kx = kk[:, None] % 32
        qx = q[None, :] % 32
        good = (np.abs(krow - qrow) <= 3) & (np.abs(kx - qx) <= 3)
        mneg[:, S_OFF[jt]:S_OFF[jt] + spn] = np.where(good, 0.0, -300.0)

    # noob folded into the O->oT copy: denominator rows 48 (e=0) and 112
    # (e=1) get the count of x-out-of-bounds keys (reference zero-pads
    # those -> exp(0) each).
    noobp = np.zeros((128, 512), np.float32)
    for qy in range(16):
        for qx in range(32):
            nb = 7.0 * (max(0, 3 - qx) + max(0, qx - 28))
            noobp[48, 32 * qy + qx] = nb
            noobp[112, 32 * qy + qx] = nb
    sel4 = np.zeros((128, 2, 4), np.float32)
    for j in range(2):
        sel4[48, j, 2 * j] = 1.0
        sel4[112, j, 2 * j + 1] = 1.0
    bsel = np.zeros((4, 2, 128), np.float32)
    for j in range(2):
        bsel[2 * j, j, 0:64] = 1.0
        bsel[2 * j + 1, j, 64:128] = 1.0
    ones1 = np.ones((1, 128), np.float32)
    identm = np.eye(128, dtype=np.float32)
    vtall = np.zeros((128, 6, 8, 16), np.float32)
    vtall[:, :, :, 0] = 1.0
    c = dict(wqk=wqk, wv=wvp, wp=wp, bp=bp, mneg=mneg, noobp=noobp,
             sel4=sel4, ones1=ones1, ident=identm, vtall=vtall)
    c["bsel"] = bsel
    return {k: np.ascontiguousarray(v.astype(BF_NP)) for k, v in c.items()}


def kernel(x, w_qkv, w_proj, b_proj, H=32, W=32):
    global LAST_RESULT
    x = np.asarray(x, np.float32)
    w_qkv = np.asarray(w_qkv, np.float32)
    w_proj = np.asarray(w_proj, np.float32)
    b_proj = np.asarray(b_proj, np.float32)
    assert x.shape == (1, NPOS, C) and int(H) == 32 and int(W) == 32

    nc = _build_nc()
    consts = _host_consts(w_qkv, w_proj, b_proj)

    x4 = x[0].reshape(T, HH, WW, C)
    in_maps = []
    for c in range(8):
        t, ry0 = c // 2, 16 * (c % 2)
        xh = np.zeros((24, WW, C), np.float32)
        lo, hi = ry0 - 3, ry0 + 21
        slo, shi = max(lo, 0), min(hi, HH)
        xh[slo - lo:shi - lo] = x4[t, slo:shi]
        xT = np.ascontiguousarray(
            xh.reshape(768, C).T.reshape(3, 128, 768).transpose(1, 0, 2)
        ).astype(BF_NP)
        in_maps.append({"xT": xT, **consts})

    trace = bool(int(os.environ.get("TRACE", "0")))
    res = run_bass_kernel_spmd(nc, in_maps, core_ids=list(range(8)),
                               trace=trace)
    LAST_RESULT = res
    out = np.concatenate([res.results[c]["out"] for c in range(8)], axis=0)
    return out.reshape(1, NPOS, C)


# revision 39
# speedup vs baseline: 1.2147x; 1.1174x over previous
"""Local (7x7 windowed) attention Trainium2 kernel, v3 (bf16).

Problem: B=1, N=4096 (T=4, H=W=32), C=384, 8 heads x hd=48, window 7x7
zero-padded (reference semantics: padded keys score exactly 0 -> weight
exp(0), value 0).

Sharding: data-parallel over positions. 8 cores; core c owns t-slice
c//2, query rows [16*(c%2), 16*(c%2)+16) (512 queries). Each core
recomputes k/v for a 3-row halo (24 rows = 768 halo positions,
zero-padded outside the image, matching the reference's zero padding).

v3 notes (each from trace evidence):
 - all matmuls bf16: 1 cyc/row at any N; fp32 ran 1.5-4 cyc/row and let
   the PE HAM clock drop to 1.2 GHz.
 - window mask = multiplicative 0/1 bf16 mask on exp(S), split between
   DVE and GpSimd (PE additive-mask matmuls were ~10k wasted rows).
 - one big exp per head instead of 3-6 small ones: ACT costs ~390 ns
   fixed per instruction on HW.
 - q and k share one 3-bank PSUM tile so evacuation is one copy per pr.
 - noob (x-out-of-bounds exp(0) count) folded into the O->SBUF copy as
   a tensor_add, denominators gathered across all 4 pr into one [8,512]
   and inverted with one reciprocal_approx_fast (DVE reciprocal costs
   6.5 ns/col; ACT Reciprocal is blocked by bass).
 - input DMAs spread across queues (sync: xT; gpsimd: consts) - 16 DMAs
   on one queue serialized ~12 us of startup in v2.
"""

import os

import numpy as np
import ml_dtypes

import concourse.bacc as bacc
import concourse.mybir as mybir
import concourse.tile as tile
from concourse.bass_utils import run_bass_kernel_spmd

F = mybir.dt.float32
R = mybir.dt.float32r
BF = mybir.dt.bfloat16
BF_NP = ml_dtypes.bfloat16

NH = 8
HD = 48
T, HH, WW = 4, 32, 32
C = 384
NPOS = T * HH * WW
SCALE = HD ** -0.5

# per key-tile jt (4 halo key rows each): (jt, ilo, span) in owned-query
# coords. jt5's span is extended 64->128 so the packed S layout has no
# uninitialized gap (the extra (k,q) pairs are out-of-window -> masked).
SPANS = [
    (0, 0, 128),
    (1, 0, 256),
    (2, 64, 320),
    (3, 192, 320),
    (4, 320, 192),
    (5, 384, 128),
]
# packed column offsets inside the [128, 1344] S/eT layout
# (bank0: jt0,jt1,jt5 = 512; bank1: jt2,jt4 = 512; bank2: jt3 = 320)
S_OFF = {0: 0, 1: 128, 5: 384, 2: 512, 4: 832, 3: 1024}
EW = 1344

_CACHE = {}
LAST_RESULT = None


def _build_nc():
    if "nc" in _CACHE:
        return _CACHE["nc"]
    nc = bacc.Bacc("TRN2", target_bir_lowering=False)

    d_xT = nc.dram_tensor("xT", [128, 3, 768], BF, kind="ExternalInput")
    d_wqk = nc.dram_tensor("wqk", [128, 3, 8, 128], BF, kind="ExternalInput")
    d_wv = nc.dram_tensor("wv", [128, 3, 384], BF, kind="ExternalInput")
    d_wp = nc.dram_tensor("wp", [128, 4, 384], BF, kind="ExternalInput")
    d_bp = nc.dram_tensor("bp", [1, 384], BF, kind="ExternalInput")
    d_mneg = nc.dram_tensor("mneg", [128, EW], BF, kind="ExternalInput")
    d_ident = nc.dram_tensor("ident", [128, 128], BF, kind="ExternalInput")
    d_noobp = nc.dram_tensor("noobp", [128, 512], BF, kind="ExternalInput")
    d_sel4 = nc.dram_tensor("sel4", [128, 2, 4], BF, kind="ExternalInput")
    d_bsel = nc.dram_tensor("bsel", [4, 2, 128], BF, kind="ExternalInput")
    d_ones1 = nc.dram_tensor("ones1", [1, 128], BF, kind="ExternalInput")
    d_vtall = nc.dram_tensor("vtall", [128, 6, 8, 16], BF, kind="ExternalInput")
    d_out = nc.dram_tensor("out", [512, 384], F, kind="ExternalOutput")

    EXP = mybir.ActivationFunctionType.Exp

    with tile.TileContext(nc) as tc:
        with tc.tile_pool(name="singles", bufs=1) as S:
            xT = S.tile([128, 3, 768], BF)
            wqk = S.tile([128, 3, 8, 128], BF)
            wv = S.tile([128, 3, 384], BF)
            wp = S.tile([128, 4, 384], BF)
            bp = S.tile([1, 384], BF)
            mneg = S.tile([128, EW], BF)
            ident = S.tile([128, 128], BF)
            noobp = S.tile([128, 512], BF)
            sel4 = S.tile([128, 2, 4], BF)
            bsel = S.tile([4, 2, 128], BF)
            ones1 = S.tile([1, 128], BF)
            qkT2 = S.tile([128, 4, 1280], BF)
            vaug = S.tile([128, 6, 8, 64], BF)
            eTall = S.tile([128, 8, EW], BF)
            nhat = S.tile([128, 4, 512], BF)

            # Spread input DMAs over four queues so transfers overlap;
            # the first matmul needs only xT[k0] + wqk[k0] (both first on
            # sync). One 786KB wqk DMA measured ~7us at ~111GB/s, so wqk
            # is split per k-slice across queues.
            nc.sync.dma_start(out=wqk[:, 0, 0:2, :], in_=d_wqk[:, 0, 0:2, :])
            nc.sync.dma_start(out=wqk[:, 0, 2:8, :], in_=d_wqk[:, 0, 2:8, :])
            nc.sync.dma_start(out=wqk[:, 2, :, :], in_=d_wqk[:, 2, :, :])
            nc.scalar.dma_start(out=xT[:, 0, :], in_=d_xT[:, 0, :])
            nc.scalar.dma_start(out=xT[:, 1, :], in_=d_xT[:, 1, :])
            nc.scalar.dma_start(out=xT[:, 2, :], in_=d_xT[:, 2, :])
            nc.scalar.dma_start(out=ident[:], in_=d_ident[:])
            nc.scalar.dma_start(out=wp[:], in_=d_wp[:])
            nc.scalar.dma_start(out=sel4[:], in_=d_sel4[:])
            nc.gpsimd.dma_start(out=wqk[:, 1, :, :], in_=d_wqk[:, 1, :, :])
            nc.gpsimd.dma_start(out=wv[:], in_=d_wv[:])
            nc.gpsimd.dma_start(out=mneg[:], in_=d_mneg[:])
            nc.gpsimd.dma_start(out=noobp[:], in_=d_noobp[:])
            nc.gpsimd.dma_start(out=bsel[:], in_=d_bsel[:])
            nc.gpsimd.dma_start(out=ones1[:], in_=d_ones1[:])
            nc.gpsimd.dma_start(out=bp[:], in_=d_bp[:])
            # vaug's denominator-ones column (col 48) + zero pad 49:63
            nc.gpsimd.dma_start(out=vaug[:, :, :, 48:64], in_=d_vtall[:])

            # ---- P1: q (owned 512) + k (halo 768) in one PSUM tile ----
            with tc.tile_pool(name="psA", bufs=2, space="PSUM") as psA:
                for pr in range(4):
                    QK = psA.tile([128, 1536], F, tag="QK")
                    for k in range(3):
                        st, sp_ = (k == 0), (k == 2)
                        nc.tensor.matmul(QK[:, 0:512], wqk[:, k, 2 * pr, :],
                                         xT[:, k, 96:608], start=st, stop=sp_)
                        nc.tensor.matmul(QK[:, 512:1024],
                                         wqk[:, k, 2 * pr + 1, :],
                                         xT[:, k, 0:512], start=st, stop=sp_)
                        nc.tensor.matmul(QK[:, 1024:1280],
                                         wqk[:, k, 2 * pr + 1, :],
                                         xT[:, k, 512:768], start=st, stop=sp_)
                    # pr 0/1 on ACT (idle until exps start), 2/3 on DVE
                    # so neither engine's queue delays the first exps
                    if pr < 2:
                        nc.scalar.copy(qkT2[:, pr, :], QK[:, 0:1280])
                    else:
                        nc.vector.tensor_copy(qkT2[:, pr, :], QK[:, 0:1280])
                for pt in range(6):
                    V = psA.tile([128, 384], F, tag="V")
                    for k in range(3):
                        nc.tensor.matmul(V[:], xT[:, k, 128 * pt:128 * pt + 128],
                                         wv[:, k, :], start=(k == 0), stop=(k == 2))
                    nc.vector.tensor_copy(
                        vaug[:, pt, :, 0:48],
                        V[:].rearrange("p (h d) -> p h d", h=8))

            # ---- P2+P3: dense score phase, then dense V phase ---------
            # psS (6 banks) and psO (2 banks) are open CONCURRENTLY so
            # the V phase does not wait on a pool-transition barrier
            # (reusing psS banks would make the first V-matmul wait for
            # the LAST pair's exp). The long uninterrupted matmul stream
            # keeps the PE HAM clock at 2.4 GHz.
            # Per head h: 6 mask matmuls (ident stationary, -300 window
            # mask) then 6 score matmuls accumulate (start=True only on
            # the first matmul touching each PSUM bank: start clears the
            # whole bank's has_written bits). exp split at column 512 so
            # the next pair's bank-0 matmuls only wait on exp part 1.
            BANK_FIRST = {0, 2, 3}
            BANK_LAST = {5, 4, 3}
            JT_BM = [(0, 0, 128), (1, 0, 256), (5, 384, 128),
                     (2, 64, 320), (4, 320, 192), (3, 192, 320)]
            import contextlib
            _stk = contextlib.ExitStack()
            sbn = _stk.enter_context(tc.tile_pool(name="sbn", bufs=2))
            sbo = _stk.enter_context(tc.tile_pool(name="sbo", bufs=2))
            oTs = {}
            for pr in range(4):
                oTs[pr] = sbo.tile([128, 512], BF, tag=f"oT{pr % 2}",
                                   name=f"oT{pr}")
            recrs = {}
            with tc.tile_pool(name="psS", bufs=1, space="PSUM") as psS, \
                 tc.tile_pool(name="psO", bufs=1, space="PSUM") as psO:
                for h in range(8):
                    pr, e = h // 2, h % 2
                    Sb = psS.tile([128, 1536], F, tag=f"S{e}", name=f"S{e}")
                    for jt, ilo, spn in JT_BM:
                        so = S_OFF[jt]
                        nc.tensor.matmul(
                            Sb[:, so:so + spn], ident[:], mneg[:, so:so + spn],
                            start=(jt in BANK_FIRST), stop=False,
                            skip_group_check=True)
                    for jt, ilo, spn in JT_BM:
                        so = S_OFF[jt]
                        nc.tensor.matmul(
                            Sb[:, so:so + spn],
                            qkT2[64 * e:64 * e + 64, pr,
                                 512 + 128 * jt:512 + 128 * (jt + 1)],
                            qkT2[64 * e:64 * e + 64, pr, ilo:ilo + spn],
                            start=False, stop=(jt in BANK_LAST),
                            skip_group_check=True)
                    nc.scalar.activation(eTall[:, h, 0:512], Sb[:, 0:512],
                                         EXP, scale=SCALE)
                    nc.scalar.activation(eTall[:, h, 512:EW], Sb[:, 512:EW],
                                         EXP, scale=SCALE)
                for pr in range(4):
                    O = psO.tile([128, 512], F, tag="O", name="O")
                    for e in range(2):
                        h = 2 * pr + e
                        for i, (jt, ilo, spn) in enumerate(SPANS):
                            nc.tensor.matmul(
                                O[64 * e:64 * e + 64, ilo:ilo + spn],
                                vaug[:, jt, h, :],
                                eTall[:, h, S_OFF[jt]:S_OFF[jt] + spn],
                                start=(i == 0), stop=(i == 5),
                                skip_group_check=True)
                    # evacuate both heads + add noob to den rows 48/112
                    nc.vector.tensor_add(oTs[pr][:], O[:], noobp[:])

            # ---- P4/P5: denominators, normalize, project --------------
            with tc.tile_pool(name="psD", bufs=2, space="PSUM") as psD, \
                 tc.tile_pool(name="psB", bufs=1, space="PSUM") as psB, \
                 tc.tile_pool(name="psP", bufs=1, space="PSUM") as psP:
                P = psP.tile([128, 4, 512], F)

                def d_mms(g):  # g = 0: pr 0,1; g = 1: pr 2,3
                    D = psD.tile([4, 512], F, tag="D", name="D")
                    for j in range(2):
                        nc.tensor.matmul(D[:], sel4[:, j, :], oTs[2 * g + j][:],
                                         start=(j == 0), stop=(j == 1),
                                         skip_group_check=True)
                    recf = sbn.tile([4, 512], F, tag="recf", name="recf")
                    nc.vector.reciprocal_approx_fast(recf[:], D[:])
                    recr = sbn.tile([4, 512], BF, tag="recr", name="recr")
                    nc.vector.tensor_copy(recr[:], recf[:])
                    recrs[g] = recr

                def bc_nhat_p5(pr):
                    Bc = psB.tile([128, 512], F, tag="Bc", name="Bc")
                    nc.tensor.matmul(Bc[:], bsel[:, pr % 2, :],
                                     recrs[pr // 2][:], start=True, stop=True)
                    nc.vector.tensor_mul(nhat[:, pr, :], oTs[pr][:], Bc[:])
                    for it in range(4):
                        nc.tensor.matmul(
                            P[:, it, 0:384],
                            nhat[:, pr, 128 * it:128 * (it + 1)],
                            wp[:, pr, :], start=(pr == 0), stop=False,
                            skip_group_check=True)

                d_mms(0)
                d_mms(1)
                for pr in range(4):
                    bc_nhat_p5(pr)
                for it in range(4):
                    nc.tensor.matmul(P[:, it, 0:384], ones1[:], bp[:],
                                     start=False, stop=True,
                                     skip_group_check=True)
                    ot = sbo.tile([128, 384], F, tag="ot", name=f"ot{it}")
                    if it % 2 == 0:
                        nc.scalar.copy(ot[:], P[:, it, 0:384])
                    else:
                        nc.vector.tensor_copy(ot[:], P[:, it, 0:384])
                    nc.gpsimd.dma_start(out=d_out[128 * it:128 * (it + 1), :],
                                        in_=ot[:])
            _stk.close()

    nc.compile()
    _CACHE["nc"] = nc
    return nc


def _host_consts(w_qkv, w_proj, b_proj):
    wqk = np.zeros((128, 3, 8, 128), np.float32)
    for k in range(3):
        rows = slice(k * 128, (k + 1) * 128)
        for pr in range(4):
            for s in range(2):  # 0 = q block, 1 = k block
                off = 384 * s
                wqk[:, k, 2 * pr + s, 0:48] = \
                    w_qkv[rows, off + 48 * (2 * pr):off + 48 * (2 * pr) + 48]
                wqk[:, k, 2 * pr + s, 64:112] = \
                    w_qkv[rows, off + 48 * (2 * pr + 1):off + 48 * (2 * pr + 1) + 48]
    wvp = np.ascontiguousarray(np.transpose(
        w_qkv[:, 768:1152].reshape(3, 128, 384), (1, 0, 2)))
    wp = np.zeros((128, 4, 384), np.float32)
    for pr in range(4):
        wp[0:48, pr, :] = w_proj[96 * pr:96 * pr + 48, :]
        wp[64:112, pr, :] = w_proj[96 * pr + 48:96 * pr + 96, :]
    bp = b_proj.reshape(1, 384)

    # additive window mask in the packed S layout: entry (k, q) of tile
    # jt is in-window iff |key_halo_row - query_halo_row| <= 3 and
    # |kx - qx| <= 3; out-of-window scores get -300 (exp -> ~1e-19).
    mneg = np.zeros((128, EW), np.float32)
    kk = np.arange(128)
    for jt, ilo, spn in SPANS:
        q = np.arange(ilo, ilo + spn)
        krow = 4 * jt + kk[:, None] // 32
        qrow = q[None, :] // 32 + 3
        kx = kk[:, None] % 32
        qx = q[None, :] % 32
        good = (np.abs(krow - qrow) <= 3) & (np.abs(kx - qx) <= 3)
        mneg[:, S_OFF[jt]:S_OFF[jt] + spn] = np.where(good, 0.0, -300.0)

    # noob folded into the O->oT copy: denominator rows 48 (e=0) and 112
    # (e=1) get the count of x-out-of-bounds keys (reference zero-pads
    # those -> exp(0) each).
    noobp = np.zeros((128, 512), np.float32)
    for qy in range(16):
        for qx in range(32):
            nb = 7.0 * (max(0, 3 - qx) + max(0, qx - 28))
            noobp[48, 32 * qy + qx] = nb
            noobp[112, 32 * qy + qx] = nb
    sel4 = np.zeros((128, 2, 4), np.float32)
    for j in range(2):
        sel4[48, j, 2 * j] = 1.0
        sel4[112, j, 2 * j + 1] = 1.0
    bsel = np.zeros((4, 2, 128), np.float32)
    for j in range(2):
        bsel[2 * j, j, 0:64] = 1.0
        bsel[2 * j + 1, j, 64:128] = 1.0
    ones1 = np.ones((1, 128), np.float32)
    identm = np.eye(128, dtype=np.float32)
    vtall = np.zeros((128, 6, 8, 16), np.float32)
    vtall[:, :, :, 0] = 1.0
    c = dict(wqk=wqk, wv=wvp, wp=wp, bp=bp, mneg=mneg, noobp=noobp,
             sel4=sel4, ones1=ones1, ident=identm, vtall=vtall)
    c["bsel"] = bsel
    return {k: np.ascontiguousarray(v.astype(BF_NP)) for k, v in c.items()}


def kernel(x, w_qkv, w_proj, b_proj, H=32, W=32):
    global LAST_RESULT
    x = np.asarray(x, np.float32)
    w_qkv = np.asarray(w_qkv, np.float32)
    w_proj = np.asarray(w_proj, np.float32)
    b_proj = np.asarray(b_proj, np.float32)
    assert x.shape == (1, NPOS, C) and int(H) == 32 and int(W) == 32

    nc = _build_nc()
    consts = _host_consts(w_qkv, w_proj, b_proj)

    x4 = x[0].reshape(T, HH, WW, C)
    in_maps = []
    for c in range(8):
        t, ry0 = c // 2, 16 * (c % 2)
        xh = np.zeros((24, WW, C), np.float32)
        lo, hi = ry0 - 3, ry0 + 21
        slo, shi = max(lo, 0), min(hi, HH)
        xh[slo - lo:shi - lo] = x4[t, slo:shi]
        xT = np.ascontiguousarray(
            xh.reshape(768, C).T.reshape(3, 128, 768).transpose(1, 0, 2)
        ).astype(BF_NP)
        in_maps.append({"xT": xT, **consts})

    trace = bool(int(os.environ.get("TRACE", "0")))
    res = run_bass_kernel_spmd(nc, in_maps, core_ids=list(range(8)),
                               trace=trace)
    LAST_RESULT = res
    out = np.concatenate([res.results[c]["out"] for c in range(8)], axis=0)
    return out.reshape(1, NPOS, C)


# revision 41
# speedup vs baseline: 1.3406x; 1.1036x over previous
"""Local (7x7 windowed) attention Trainium2 kernel, v3 (bf16).

Problem: B=1, N=4096 (T=4, H=W=32), C=384, 8 heads x hd=48, window 7x7
zero-padded (reference semantics: padded keys score exactly 0 -> weight
exp(0), value 0).

Sharding: data-parallel over positions. 8 cores; core c owns t-slice
c//2, query rows [16*(c%2), 16*(c%2)+16) (512 queries). Each core
recomputes k/v for a 3-row halo (24 rows = 768 halo positions,
zero-padded outside the image, matching the reference's zero padding).

v3 notes (each from trace evidence):
 - all matmuls bf16: 1 cyc/row at any N; fp32 ran 1.5-4 cyc/row and let
   the PE HAM clock drop to 1.2 GHz.
 - window mask = multiplicative 0/1 bf16 mask on exp(S), split between
   DVE and GpSimd (PE additive-mask matmuls were ~10k wasted rows).
 - one big exp per head instead of 3-6 small ones: ACT costs ~390 ns
   fixed per instruction on HW.
 - q and k share one 3-bank PSUM tile so evacuation is one copy per pr.
 - noob (x-out-of-bounds exp(0) count) folded into the O->SBUF copy as
   a tensor_add, denominators gathered across all 4 pr into one [8,512]
   and inverted with one reciprocal_approx_fast (DVE reciprocal costs
   6.5 ns/col; ACT Reciprocal is blocked by bass).
 - input DMAs spread across queues (sync: xT; gpsimd: consts) - 16 DMAs
   on one queue serialized ~12 us of startup in v2.
"""

import os

import numpy as np
import ml_dtypes

import concourse.bacc as bacc
import concourse.mybir as mybir
import concourse.tile as tile
from concourse.bass_utils import run_bass_kernel_spmd

F = mybir.dt.float32
R = mybir.dt.float32r
BF = mybir.dt.bfloat16
BF_NP = ml_dtypes.bfloat16

NH = 8
HD = 48
T, HH, WW = 4, 32, 32
C = 384
NPOS = T * HH * WW
SCALE = HD ** -0.5

# per key-tile jt (4 halo key rows each): (jt, ilo, span) in owned-query
# coords. jt5's span is extended 64->128 so the packed S layout has no
# uninitialized gap (the extra (k,q) pairs are out-of-window -> masked).
SPANS = [
    (0, 0, 128),
    (1, 0, 256),
    (2, 64, 320),
    (3, 192, 320),
    (4, 320, 192),
    (5, 384, 128),
]
# packed column offsets inside the [128, 1344] S/eT layout
# (bank0: jt0,jt1,jt5 = 512; bank1: jt2,jt4 = 512; bank2: jt3 = 320)
S_OFF = {0: 0, 1: 128, 5: 384, 2: 512, 4: 832, 3: 1024}
EW = 1344

_CACHE = {}
LAST_RESULT = None


def _build_nc():
    if "nc" in _CACHE:
        return _CACHE["nc"]
    nc = bacc.Bacc("TRN2", target_bir_lowering=False)

    d_xT = nc.dram_tensor("xT", [128, 3, 768], BF, kind="ExternalInput")
    d_wqk = nc.dram_tensor("wqk", [128, 3, 8, 128], BF, kind="ExternalInput")
    d_wv = nc.dram_tensor("wv", [128, 3, 384], BF, kind="ExternalInput")
    d_wp = nc.dram_tensor("wp", [128, 4, 384], BF, kind="ExternalInput")
    d_bp = nc.dram_tensor("bp", [1, 384], BF, kind="ExternalInput")
    d_mneg = nc.dram_tensor("mneg", [128, EW], BF, kind="ExternalInput")
    d_ident = nc.dram_tensor("ident", [128, 128], BF, kind="ExternalInput")
    d_noobp = nc.dram_tensor("noobp", [128, 512], BF, kind="ExternalInput")
    d_sel4 = nc.dram_tensor("sel4", [128, 2, 4], BF, kind="ExternalInput")
    d_bsel = nc.dram_tensor("bsel", [4, 2, 128], BF, kind="ExternalInput")
    d_ones1 = nc.dram_tensor("ones1", [1, 128], BF, kind="ExternalInput")
    d_vtall = nc.dram_tensor("vtall", [128, 6, 8, 16], BF, kind="ExternalInput")
    d_out = nc.dram_tensor("out", [512, 384], F, kind="ExternalOutput")

    EXP = mybir.ActivationFunctionType.Exp

    with tile.TileContext(nc) as tc:
        with tc.tile_pool(name="singles", bufs=1) as S:
            xT = S.tile([128, 3, 768], BF)
            wqk = S.tile([128, 3, 8, 128], BF)
            wv = S.tile([128, 3, 384], BF)
            wp = S.tile([128, 4, 384], BF)
            bp = S.tile([1, 384], BF)
            mneg = S.tile([128, EW], BF)
            ident = S.tile([128, 128], BF)
            noobp = S.tile([128, 512], BF)
            sel4 = S.tile([128, 2, 4], BF)
            bsel = S.tile([4, 2, 128], BF)
            ones1 = S.tile([1, 128], BF)
            qkT2 = S.tile([128, 4, 1280], BF)
            vaug = S.tile([128, 6, 8, 64], BF)
            eTall = S.tile([128, 8, EW], BF)
            nhat = S.tile([128, 4, 512], BF)

            # Spread input DMAs over four queues so transfers overlap;
            # the first matmul needs only xT[k0] + wqk[k0] (both first on
            # sync). One 786KB wqk DMA measured ~7us at ~111GB/s, so wqk
            # is split per k-slice across queues.
            nc.sync.dma_start(out=wqk[:, 0, 0:2, :], in_=d_wqk[:, 0, 0:2, :])
            nc.sync.dma_start(out=wqk[:, 0, 2:8, :], in_=d_wqk[:, 0, 2:8, :])
            nc.sync.dma_start(out=wqk[:, 2, :, :], in_=d_wqk[:, 2, :, :])
            nc.scalar.dma_start(out=xT[:, 0, :], in_=d_xT[:, 0, :])
            nc.scalar.dma_start(out=xT[:, 1, :], in_=d_xT[:, 1, :])
            nc.scalar.dma_start(out=xT[:, 2, :], in_=d_xT[:, 2, :])
            nc.scalar.dma_start(out=ident[:], in_=d_ident[:])
            nc.scalar.dma_start(out=wp[:], in_=d_wp[:])
            nc.scalar.dma_start(out=sel4[:], in_=d_sel4[:])
            nc.gpsimd.dma_start(out=wqk[:, 1, :, :], in_=d_wqk[:, 1, :, :])
            nc.gpsimd.dma_start(out=wv[:], in_=d_wv[:])
            nc.gpsimd.dma_start(out=mneg[:], in_=d_mneg[:])
            nc.gpsimd.dma_start(out=noobp[:], in_=d_noobp[:])
            nc.gpsimd.dma_start(out=bsel[:], in_=d_bsel[:])
            nc.gpsimd.dma_start(out=ones1[:], in_=d_ones1[:])
            nc.gpsimd.dma_start(out=bp[:], in_=d_bp[:])
            # vaug's denominator-ones column (col 48) + zero pad 49:63
            nc.gpsimd.dma_start(out=vaug[:, :, :, 48:64], in_=d_vtall[:])

            # ---- P1: q (owned 512) + k (halo 768) in one PSUM tile ----
            with tc.tile_pool(name="psA", bufs=2, space="PSUM") as psA:
                for pr in range(4):
                    QK = psA.tile([128, 1536], F, tag="QK")
                    for k in range(3):
                        st, sp_ = (k == 0), (k == 2)
                        nc.tensor.matmul(QK[:, 0:512], wqk[:, k, 2 * pr, :],
                                         xT[:, k, 96:608], start=st, stop=sp_)
                        nc.tensor.matmul(QK[:, 512:1024],
                                         wqk[:, k, 2 * pr + 1, :],
                                         xT[:, k, 0:512], start=st, stop=sp_)
                        nc.tensor.matmul(QK[:, 1024:1280],
                                         wqk[:, k, 2 * pr + 1, :],
                                         xT[:, k, 512:768], start=st, stop=sp_)
                    # pr 0/1 on ACT (idle until exps start), 2/3 on DVE
                    # so neither engine's queue delays the first exps
                    if pr < 2:
                        nc.scalar.copy(qkT2[:, pr, :], QK[:, 0:1280])
                    else:
                        nc.vector.tensor_copy(qkT2[:, pr, :], QK[:, 0:1280])
                for pt in range(6):
                    V = psA.tile([128, 384], F, tag="V")
                    for k in range(3):
                        nc.tensor.matmul(V[:], xT[:, k, 128 * pt:128 * pt + 128],
                                         wv[:, k, :], start=(k == 0), stop=(k == 2))
                    nc.vector.tensor_copy(
                        vaug[:, pt, :, 0:48],
                        V[:].rearrange("p (h d) -> p h d", h=8))

            # ---- P2+P3: dense score phase, then dense V phase ---------
            # psS (6 banks) and psO (2 banks) are open CONCURRENTLY so
            # the V phase does not wait on a pool-transition barrier
            # (reusing psS banks would make the first V-matmul wait for
            # the LAST pair's exp). The long uninterrupted matmul stream
            # keeps the PE HAM clock at 2.4 GHz.
            # Per head h: 6 mask matmuls (ident stationary, -300 window
            # mask) then 6 score matmuls accumulate (start=True only on
            # the first matmul touching each PSUM bank: start clears the
            # whole bank's has_written bits). exp split at column 512 so
            # the next pair's bank-0 matmuls only wait on exp part 1.
            BANK_FIRST = {0, 2, 3}
            BANK_LAST = {5, 4, 3}
            JT_BM = [(0, 0, 128), (1, 0, 256), (5, 384, 128),
                     (2, 64, 320), (4, 320, 192), (3, 192, 320)]
            import contextlib
            _stk = contextlib.ExitStack()
            sbn = _stk.enter_context(tc.tile_pool(name="sbn", bufs=2))
            sbo = _stk.enter_context(tc.tile_pool(name="sbo", bufs=2))
            oTs = {}
            for pr in range(4):
                oTs[pr] = sbo.tile([128, 512], BF, tag=f"oT{pr % 2}",
                                   name=f"oT{pr}")
            recrs = {}

            def d_mms(psD, g):  # g = 0: pr 0,1; g = 1: pr 2,3
                D = psD.tile([4, 512], F, tag="D", name="D")
                for j in range(2):
                    nc.tensor.matmul(D[:], sel4[:, j, :], oTs[2 * g + j][:],
                                     start=(j == 0), stop=(j == 1),
                                     skip_group_check=True)
                recf = sbn.tile([4, 512], F, tag="recf", name="recf")
                nc.vector.reciprocal_approx_fast(recf[:], D[:])
                recr = sbn.tile([4, 512], BF, tag="recr", name="recr")
                nc.vector.tensor_copy(recr[:], recf[:])
                recrs[g] = recr

            with tc.tile_pool(name="psS", bufs=1, space="PSUM") as psS, \
                 tc.tile_pool(name="psO", bufs=1, space="PSUM") as psO, \
                 tc.tile_pool(name="psD", bufs=1, space="PSUM") as psD:
                for h in range(8):
                    pr, e = h // 2, h % 2
                    Sb = psS.tile([128, 1536], F, tag=f"S{e}", name=f"S{e}")
                    for jt, ilo, spn in JT_BM:
                        so = S_OFF[jt]
                        nc.tensor.matmul(
                            Sb[:, so:so + spn], ident[:], mneg[:, so:so + spn],
                            start=(jt in BANK_FIRST), stop=False,
                            skip_group_check=True)
                    for jt, ilo, spn in JT_BM:
                        so = S_OFF[jt]
                        nc.tensor.matmul(
                            Sb[:, so:so + spn],
                            qkT2[64 * e:64 * e + 64, pr,
                                 512 + 128 * jt:512 + 128 * (jt + 1)],
                            qkT2[64 * e:64 * e + 64, pr, ilo:ilo + spn],
                            start=False, stop=(jt in BANK_LAST),
                            skip_group_check=True)
                    nc.scalar.activation(eTall[:, h, 0:512], Sb[:, 0:512],
                                         EXP, scale=SCALE)
                    nc.scalar.activation(eTall[:, h, 512:EW], Sb[:, 512:EW],
                                         EXP, scale=SCALE)
                for pr in range(4):
                    O = psO.tile([128, 512], F, tag="O", name="O")
                    for e in range(2):
                        h = 2 * pr + e
                        for i, (jt, ilo, spn) in enumerate(SPANS):
                            nc.tensor.matmul(
                                O[64 * e:64 * e + 64, ilo:ilo + spn],
                                vaug[:, jt, h, :],
                                eTall[:, h, S_OFF[jt]:S_OFF[jt] + spn],
                                start=(i == 0), stop=(i == 5),
                                skip_group_check=True)
                    # evacuate both heads + add noob to den rows 48/112
                    nc.vector.tensor_add(oTs[pr][:], O[:], noobp[:])
                    if pr % 2 == 1:
                        # reciprocal of pr-pair denominators runs on DVE
                        # during the remaining V-matmuls
                        d_mms(psD, pr // 2)

            # ---- P4/P5: normalize + project ---------------------------
            with tc.tile_pool(name="psB", bufs=2, space="PSUM") as psB, \
                 tc.tile_pool(name="psP", bufs=1, space="PSUM") as psP:
                P = psP.tile([128, 4, 512], F)

                def bc(pr):
                    Bc = psB.tile([128, 512], F, tag="Bc", name=f"Bc{pr}")
                    nc.tensor.matmul(Bc[:], bsel[:, pr % 2, :],
                                     recrs[pr // 2][:], start=True, stop=True)
                    return Bc

                def nhat_p5(pr, Bc):
                    nc.vector.tensor_mul(nhat[:, pr, :], oTs[pr][:], Bc[:])
                    for it in range(4):
                        nc.tensor.matmul(
                            P[:, it, 0:384],
                            nhat[:, pr, 128 * it:128 * (it + 1)],
                            wp[:, pr, :], start=(pr == 0), stop=False,
                            skip_group_check=True)

                Bc0 = bc(0)
                Bc1 = bc(1)
                nhat_p5(0, Bc0)
                Bc2 = bc(2)
                nhat_p5(1, Bc1)
                Bc3 = bc(3)
                nhat_p5(2, Bc2)
                nhat_p5(3, Bc3)
                for it in range(4):
                    nc.tensor.matmul(P[:, it, 0:384], ones1[:], bp[:],
                                     start=False, stop=True,
                                     skip_group_check=True)
                    ot = sbo.tile([128, 384], F, tag="ot", name=f"ot{it}")
                    if it % 2 == 0:
                        nc.scalar.copy(ot[:], P[:, it, 0:384])
                    else:
                        nc.vector.tensor_copy(ot[:], P[:, it, 0:384])
                    nc.gpsimd.dma_start(out=d_out[128 * it:128 * (it + 1), :],
                                        in_=ot[:])
            _stk.close()

    nc.compile()
    _CACHE["nc"] = nc
    return nc


def _host_consts(w_qkv, w_proj, b_proj):
    wqk = np.zeros((128, 3, 8, 128), np.float32)
    for k in range(3):
        rows = slice(k * 128, (k + 1) * 128)
        for pr in range(4):
            for s in range(2):  # 0 = q block, 1 = k block
                off = 384 * s
                wqk[:, k, 2 * pr + s, 0:48] = \
                    w_qkv[rows, off + 48 * (2 * pr):off + 48 * (2 * pr) + 48]
                wqk[:, k, 2 * pr + s, 64:112] = \
                    w_qkv[rows, off + 48 * (2 * pr + 1):off + 48 * (2 * pr + 1) + 48]
    wvp = np.ascontiguousarray(np.transpose(
        w_qkv[:, 768:1152].reshape(3, 128, 384), (1, 0, 2)))
    wp = np.zeros((128, 4, 384), np.float32)
    for pr in range(4):
        wp[0:48, pr, :] = w_proj[96 * pr:96 * pr + 48, :]
        wp[64:112, pr, :] = w_proj[96 * pr + 48:96 * pr + 96, :]
    bp = b_proj.reshape(1, 384)

    # additive window mask in the packed S layout: entry (k, q) of tile
    # jt is in-window iff |key_halo_row - query_halo_row| <= 3 and
    # |kx - qx| <= 3; out-of-window scores get -300 (exp -> ~1e-19).
    mneg = np.zeros((128, EW), np.float32)
    kk = np.arange(128)
    for jt, ilo, spn in SPANS:
        q = np.arange(ilo, ilo + spn)
        krow = 4 * jt + kk[:, None] // 32
        qrow = q[None, :] // 32 + 3
        kx = kk[:, None] % 32
        qx = q[None, :] % 32
        good = (np.abs(krow - qrow) <= 3) & (np.abs(kx - qx) <= 3)
        mneg[:, S_OFF[jt]:S_OFF[jt] + spn] = np.where(good, 0.0, -300.0)

    # noob folded into the O->oT copy: denominator rows 48 (e=0) and 112
    # (e=1) get the count of x-out-of-bounds keys (reference zero-pads
    # those -> exp(0) each).
    noobp = np.zeros((128, 512), np.float32)
    for qy in range(16):
        for qx in range(32):
            nb = 7.0 * (max(0, 3 - qx) + max(0, qx - 28))
            noobp[48, 32 * qy + qx] = nb
            noobp[112, 32 * qy + qx] = nb
    sel4 = np.zeros((128, 2, 4), np.float32)
    for j in range(2):
        sel4[48, j, 2 * j] = 1.0
        sel4[112, j, 2 * j + 1] = 1.0
    bsel = np.zeros((4, 2, 128), np.float32)
    for j in range(2):
        bsel[2 * j, j, 0:64] = 1.0
        bsel[2 * j + 1, j, 64:128] = 1.0
    ones1 = np.ones((1, 128), np.float32)
    identm = np.eye(128, dtype=np.float32)
    vtall = np.zeros((128, 6, 8, 16), np.float32)
    vtall[:, :, :, 0] = 1.0
    c = dict(wqk=wqk, wv=wvp, wp=wp, bp=bp, mneg=mneg, noobp=noobp,
             sel4=sel4, ones1=ones1, ident=identm, vtall=vtall)
    c["bsel"] = bsel
    return {k: np.ascontiguousarray(v.astype(BF_NP)) for k, v in c.items()}


def kernel(x, w_qkv, w_proj, b_proj, H=32, W=32):
    global LAST_RESULT
    x = np.asarray(x, np.float32)
    w_qkv = np.asarray(w_qkv, np.float32)
    w_proj = np.asarray(w_proj, np.float32)
    b_proj = np.asarray(b_proj, np.float32)
    assert x.shape == (1, NPOS, C) and int(H) == 32 and int(W) == 32

    nc = _build_nc()
    consts = _host_consts(w_qkv, w_proj, b_proj)

    x4 = x[0].reshape(T, HH, WW, C)
    in_maps = []
    for c in range(8):
        t, ry0 = c // 2, 16 * (c % 2)
        xh = np.zeros((24, WW, C), np.float32)
        lo, hi = ry0 - 3, ry0 + 21
        slo, shi = max(lo, 0), min(hi, HH)
        xh[slo - lo:shi - lo] = x4[t, slo:shi]
        xT = np.ascontiguousarray(
            xh.reshape(768, C).T.reshape(3, 128, 768).transpose(1, 0, 2)
        ).astype(BF_NP)
        in_maps.append({"xT": xT, **consts})

    trace = bool(int(os.environ.get("TRACE", "0")))
    res = run_bass_kernel_spmd(nc, in_maps, core_ids=list(range(8)),
                               trace=trace)
    LAST_RESULT = res
    out = np.concatenate([res.results[c]["out"] for c in range(8)], axis=0)
    return out.reshape(1, NPOS, C)


# revision 44
# speedup vs baseline: 1.3566x; 1.0119x over previous
"""Local (7x7 windowed) attention Trainium2 kernel, v3 (bf16).

Problem: B=1, N=4096 (T=4, H=W=32), C=384, 8 heads x hd=48, window 7x7
zero-padded (reference semantics: padded keys score exactly 0 -> weight
exp(0), value 0).

Sharding: data-parallel over positions. 8 cores; core c owns t-slice
c//2, query rows [16*(c%2), 16*(c%2)+16) (512 queries). Each core
recomputes k/v for a 3-row halo (24 rows = 768 halo positions,
zero-padded outside the image, matching the reference's zero padding).

v3 notes (each from trace evidence):
 - all matmuls bf16: 1 cyc/row at any N; fp32 ran 1.5-4 cyc/row and let
   the PE HAM clock drop to 1.2 GHz.
 - window mask = multiplicative 0/1 bf16 mask on exp(S), split between
   DVE and GpSimd (PE additive-mask matmuls were ~10k wasted rows).
 - one big exp per head instead of 3-6 small ones: ACT costs ~390 ns
   fixed per instruction on HW.
 - q and k share one 3-bank PSUM tile so evacuation is one copy per pr.
 - noob (x-out-of-bounds exp(0) count) folded into the O->SBUF copy as
   a tensor_add, denominators gathered across all 4 pr into one [8,512]
   and inverted with one reciprocal_approx_fast (DVE reciprocal costs
   6.5 ns/col; ACT Reciprocal is blocked by bass).
 - input DMAs spread across queues (sync: xT; gpsimd: consts) - 16 DMAs
   on one queue serialized ~12 us of startup in v2.
"""

import os

import numpy as np
import ml_dtypes

import concourse.bacc as bacc
import concourse.mybir as mybir
import concourse.tile as tile
from concourse.bass_utils import run_bass_kernel_spmd

F = mybir.dt.float32
R = mybir.dt.float32r
BF = mybir.dt.bfloat16
BF_NP = ml_dtypes.bfloat16

NH = 8
HD = 48
T, HH, WW = 4, 32, 32
C = 384
NPOS = T * HH * WW
SCALE = HD ** -0.5

# per key-tile jt (4 halo key rows each): (jt, ilo, span) in owned-query
# coords. jt5's span is extended 64->128 so the packed S layout has no
# uninitialized gap (the extra (k,q) pairs are out-of-window -> masked).
SPANS = [
    (0, 0, 128),
    (1, 0, 256),
    (2, 64, 320),
    (3, 192, 320),
    (4, 320, 192),
    (5, 384, 128),
]
# packed column offsets inside the [128, 1344] S/eT layout
# (bank0: jt0,jt1,jt5 = 512; bank1: jt2,jt4 = 512; bank2: jt3 = 320)
S_OFF = {0: 0, 1: 128, 5: 384, 2: 512, 4: 832, 3: 1024}
EW = 1344

_CACHE = {}
LAST_RESULT = None


def _build_nc():
    if "nc" in _CACHE:
        return _CACHE["nc"]
    nc = bacc.Bacc("TRN2", target_bir_lowering=False)

    d_xT = nc.dram_tensor("xT", [128, 3, 768], BF, kind="ExternalInput")
    d_wqk = nc.dram_tensor("wqk", [128, 3, 8, 128], BF, kind="ExternalInput")
    d_wv = nc.dram_tensor("wv", [128, 3, 384], BF, kind="ExternalInput")
    d_wp = nc.dram_tensor("wp", [128, 4, 384], BF, kind="ExternalInput")
    d_bp = nc.dram_tensor("bp", [1, 384], BF, kind="ExternalInput")
    d_mneg = nc.dram_tensor("mneg", [128, EW], BF, kind="ExternalInput")
    d_ident = nc.dram_tensor("ident", [128, 128], BF, kind="ExternalInput")
    d_noobp = nc.dram_tensor("noobp", [128, 512], BF, kind="ExternalInput")
    d_sel4 = nc.dram_tensor("sel4", [128, 2, 4], BF, kind="ExternalInput")
    d_bsel = nc.dram_tensor("bsel", [4, 2, 128], BF, kind="ExternalInput")
    d_ones1 = nc.dram_tensor("ones1", [1, 128], BF, kind="ExternalInput")
    d_vtall = nc.dram_tensor("vtall", [128, 6, 8, 16], BF, kind="ExternalInput")
    d_out = nc.dram_tensor("out", [512, 384], F, kind="ExternalOutput")

    EXP = mybir.ActivationFunctionType.Exp

    with tile.TileContext(nc) as tc:
        with tc.tile_pool(name="singles", bufs=1) as S:
            xT = S.tile([128, 3, 768], BF)
            wqk = S.tile([128, 3, 8, 128], BF)
            wv = S.tile([128, 3, 384], BF)
            wp = S.tile([128, 4, 384], BF)
            bp = S.tile([1, 384], BF)
            mneg = S.tile([128, EW], BF)
            ident = S.tile([128, 128], BF)
            noobp = S.tile([128, 512], BF)
            sel4 = S.tile([128, 2, 4], BF)
            bsel = S.tile([4, 2, 128], BF)
            ones1 = S.tile([1, 128], BF)
            qkT2 = S.tile([128, 4, 1280], BF)
            vaug = S.tile([128, 6, 8, 64], BF)
            eTall = S.tile([128, 8, EW], BF)
            nhat = S.tile([128, 4, 512], BF)

            # Spread input DMAs over four queues so transfers overlap;
            # the first matmul needs only xT[k0] + wqk[k0] (both first on
            # sync). One 786KB wqk DMA measured ~7us at ~111GB/s, so wqk
            # is split per k-slice across queues.
            nc.sync.dma_start(out=wqk[:, 0, 0:2, :], in_=d_wqk[:, 0, 0:2, :])
            nc.sync.dma_start(out=xT[:, 0, 384:768], in_=d_xT[:, 0, 384:768])
            nc.sync.dma_start(out=wqk[:, 0, 2:8, :], in_=d_wqk[:, 0, 2:8, :])
            nc.sync.dma_start(out=wqk[:, 2, :, :], in_=d_wqk[:, 2, :, :])
            nc.scalar.dma_start(out=xT[:, 0, 0:384], in_=d_xT[:, 0, 0:384])
            nc.scalar.dma_start(out=xT[:, 1, :], in_=d_xT[:, 1, :])
            nc.scalar.dma_start(out=xT[:, 2, :], in_=d_xT[:, 2, :])
            nc.scalar.dma_start(out=ident[:], in_=d_ident[:])
            nc.scalar.dma_start(out=wp[:], in_=d_wp[:])
            nc.scalar.dma_start(out=sel4[:], in_=d_sel4[:])
            nc.gpsimd.dma_start(out=wqk[:, 1, :, :], in_=d_wqk[:, 1, :, :])
            nc.gpsimd.dma_start(out=wv[:], in_=d_wv[:])
            nc.gpsimd.dma_start(out=mneg[:], in_=d_mneg[:])
            nc.gpsimd.dma_start(out=noobp[:], in_=d_noobp[:])
            nc.gpsimd.dma_start(out=bsel[:], in_=d_bsel[:])
            nc.gpsimd.dma_start(out=ones1[:], in_=d_ones1[:])
            nc.gpsimd.dma_start(out=bp[:], in_=d_bp[:])
            # vaug's denominator-ones column (col 48) + zero pad 49:63
            nc.gpsimd.dma_start(out=vaug[:, :, :, 48:64], in_=d_vtall[:])

            # ---- P1: q (owned 512) + k (halo 768) in one PSUM tile ----
            with tc.tile_pool(name="psA", bufs=2, space="PSUM") as psA:
                for pr in range(4):
                    QK = psA.tile([128, 1536], F, tag="QK")
                    for k in range(3):
                        st, sp_ = (k == 0), (k == 2)
                        nc.tensor.matmul(QK[:, 0:512], wqk[:, k, 2 * pr, :],
                                         xT[:, k, 96:608], start=st, stop=sp_)
                        nc.tensor.matmul(QK[:, 512:1024],
                                         wqk[:, k, 2 * pr + 1, :],
                                         xT[:, k, 0:512], start=st, stop=sp_)
                        nc.tensor.matmul(QK[:, 1024:1280],
                                         wqk[:, k, 2 * pr + 1, :],
                                         xT[:, k, 512:768], start=st, stop=sp_)
                    # pr 0/1 on ACT (idle until exps start), 2/3 on DVE
                    # so neither engine's queue delays the first exps
                    if pr < 2:
                        nc.scalar.copy(qkT2[:, pr, :], QK[:, 0:1280])
                    else:
                        nc.vector.tensor_copy(qkT2[:, pr, :], QK[:, 0:1280])
                for pt in range(6):
                    V = psA.tile([128, 384], F, tag="V")
                    for k in range(3):
                        nc.tensor.matmul(V[:], xT[:, k, 128 * pt:128 * pt + 128],
                                         wv[:, k, :], start=(k == 0), stop=(k == 2))
                    nc.vector.tensor_copy(
                        vaug[:, pt, :, 0:48],
                        V[:].rearrange("p (h d) -> p h d", h=8))

            # ---- P2+P3: dense score phase, then dense V phase ---------
            # psS (6 banks) and psO (2 banks) are open CONCURRENTLY so
            # the V phase does not wait on a pool-transition barrier
            # (reusing psS banks would make the first V-matmul wait for
            # the LAST pair's exp). The long uninterrupted matmul stream
            # keeps the PE HAM clock at 2.4 GHz.
            # Per head h: 6 mask matmuls (ident stationary, -300 window
            # mask) then 6 score matmuls accumulate (start=True only on
            # the first matmul touching each PSUM bank: start clears the
            # whole bank's has_written bits). exp split at column 512 so
            # the next pair's bank-0 matmuls only wait on exp part 1.
            BANK_FIRST = {0, 2, 3}
            BANK_LAST = {5, 4, 3}
            JT_BM = [(0, 0, 128), (1, 0, 256), (5, 384, 128),
                     (2, 64, 320), (4, 320, 192), (3, 192, 320)]
            import contextlib
            _stk = contextlib.ExitStack()
            sbn = _stk.enter_context(tc.tile_pool(name="sbn", bufs=2))
            sbo = _stk.enter_context(tc.tile_pool(name="sbo", bufs=2))
            oTs = {}
            for pr in range(4):
                oTs[pr] = sbo.tile([128, 512], BF, tag=f"oT{pr % 2}",
                                   name=f"oT{pr}")
            recrs = {}

            def d_mms(psD, g):  # g = 0: pr 0,1; g = 1: pr 2,3
                D = psD.tile([4, 512], F, tag="D", name="D")
                for j in range(2):
                    nc.tensor.matmul(D[:], sel4[:, j, :], oTs[2 * g + j][:],
                                     start=(j == 0), stop=(j == 1),
                                     skip_group_check=True)
                recf = sbn.tile([4, 512], F, tag="recf", name="recf")
                nc.vector.reciprocal_approx_fast(recf[:], D[:])
                recr = sbn.tile([4, 512], BF, tag="recr", name="recr")
                nc.vector.tensor_copy(recr[:], recf[:])
                recrs[g] = recr

            with tc.tile_pool(name="psS", bufs=1, space="PSUM") as psS, \
                 tc.tile_pool(name="psO", bufs=1, space="PSUM") as psO, \
                 tc.tile_pool(name="psD", bufs=1, space="PSUM") as psD:
                for h in range(8):
                    pr, e = h // 2, h % 2
                    Sb = psS.tile([128, 1536], F, tag=f"S{e}", name=f"S{e}")
                    for jt, ilo, spn in JT_BM:
                        so = S_OFF[jt]
                        nc.tensor.matmul(
                            Sb[:, so:so + spn], ident[:], mneg[:, so:so + spn],
                            start=(jt in BANK_FIRST), stop=False,
                            skip_group_check=True)
                    for jt, ilo, spn in JT_BM:
                        so = S_OFF[jt]
                        nc.tensor.matmul(
                            Sb[:, so:so + spn],
                            qkT2[64 * e:64 * e + 64, pr,
                                 512 + 128 * jt:512 + 128 * (jt + 1)],
                            qkT2[64 * e:64 * e + 64, pr, ilo:ilo + spn],
                            start=False, stop=(jt in BANK_LAST),
                            skip_group_check=True)
                    nc.scalar.activation(eTall[:, h, 0:512], Sb[:, 0:512],
                                         EXP, scale=SCALE)
                    nc.scalar.activation(eTall[:, h, 512:EW], Sb[:, 512:EW],
                                         EXP, scale=SCALE)
                for pr in range(4):
                    O = psO.tile([128, 512], F, tag="O", name="O")
                    for e in range(2):
                        h = 2 * pr + e
                        for i, (jt, ilo, spn) in enumerate(SPANS):
                            nc.tensor.matmul(
                                O[64 * e:64 * e + 64, ilo:ilo + spn],
                                vaug[:, jt, h, :],
                                eTall[:, h, S_OFF[jt]:S_OFF[jt] + spn],
                                start=(i == 0), stop=(i == 5),
                                skip_group_check=True)
                    # evacuate both heads + add noob to den rows 48/112
                    nc.vector.tensor_add(oTs[pr][:], O[:], noobp[:])
                    if pr % 2 == 1:
                        # reciprocal of pr-pair denominators runs on DVE
                        # during the remaining V-matmuls
                        d_mms(psD, pr // 2)

            # ---- P4/P5: normalize + project ---------------------------
            with tc.tile_pool(name="psB", bufs=2, space="PSUM") as psB, \
                 tc.tile_pool(name="psP", bufs=1, space="PSUM") as psP:
                P = psP.tile([128, 4, 512], F)

                def bc(pr):
                    Bc = psB.tile([128, 512], F, tag="Bc", name=f"Bc{pr}")
                    nc.tensor.matmul(Bc[:], bsel[:, pr % 2, :],
                                     recrs[pr // 2][:], start=True, stop=True)
                    return Bc

                def nhat_p5(pr, Bc):
                    # halves so P5 of it 0/1 starts after the first half
                    nc.vector.tensor_mul(nhat[:, pr, 0:256],
                                         oTs[pr][:, 0:256], Bc[:, 0:256])
                    nc.vector.tensor_mul(nhat[:, pr, 256:512],
                                         oTs[pr][:, 256:512], Bc[:, 256:512])
                    for it in range(4):
                        nc.tensor.matmul(
                            P[:, it, 0:384],
                            nhat[:, pr, 128 * it:128 * (it + 1)],
                            wp[:, pr, :], start=(pr == 0), stop=False,
                            skip_group_check=True)

                Bc0 = bc(0)
                Bc1 = bc(1)
                nhat_p5(0, Bc0)
                Bc2 = bc(2)
                nhat_p5(1, Bc1)
                Bc3 = bc(3)
                nhat_p5(2, Bc2)
                nhat_p5(3, Bc3)
                for it in range(4):
                    nc.tensor.matmul(P[:, it, 0:384], ones1[:], bp[:],
                                     start=False, stop=True,
                                     skip_group_check=True)
                    ot = sbo.tile([128, 384], F, tag="ot", name=f"ot{it}")
                    if it % 2 == 0:
                        nc.scalar.copy(ot[:], P[:, it, 0:384])
                    else:
                        nc.vector.tensor_copy(ot[:], P[:, it, 0:384])
                    nc.gpsimd.dma_start(out=d_out[128 * it:128 * (it + 1), :],
                                        in_=ot[:])
            _stk.close()

    nc.compile()
    _CACHE["nc"] = nc
    return nc


def _host_consts(w_qkv, w_proj, b_proj):
    wqk = np.zeros((128, 3, 8, 128), np.float32)
    for k in range(3):
        rows = slice(k * 128, (k + 1) * 128)
        for pr in range(4):
            for s in range(2):  # 0 = q block, 1 = k block
                off = 384 * s
                wqk[:, k, 2 * pr + s, 0:48] = \
                    w_qkv[rows, off + 48 * (2 * pr):off + 48 * (2 * pr) + 48]
                wqk[:, k, 2 * pr + s, 64:112] = \
                    w_qkv[rows, off + 48 * (2 * pr + 1):off + 48 * (2 * pr + 1) + 48]
    wvp = np.ascontiguousarray(np.transpose(
        w_qkv[:, 768:1152].reshape(3, 128, 384), (1, 0, 2)))
    wp = np.zeros((128, 4, 384), np.float32)
    for pr in range(4):
        wp[0:48, pr, :] = w_proj[96 * pr:96 * pr + 48, :]
        wp[64:112, pr, :] = w_proj[96 * pr + 48:96 * pr + 96, :]
    bp = b_proj.reshape(1, 384)

    # additive window mask in the packed S layout: entry (k, q) of tile
    # jt is in-window iff |key_halo_row - query_halo_row| <= 3 and
    # |kx - qx| <= 3; out-of-window scores get -300 (exp -> ~1e-19).
    mneg = np.zeros((128, EW), np.float32)
    kk = np.arange(128)
    for jt, ilo, spn in SPANS:
        q = np.arange(ilo, ilo + spn)
        krow = 4 * jt + kk[:, None] // 32
        qrow = q[None, :] // 32 + 3
        kx = kk[:, None] % 32
        qx = q[None, :] % 32
        good = (np.abs(krow - qrow) <= 3) & (np.abs(kx - qx) <= 3)
        mneg[:, S_OFF[jt]:S_OFF[jt] + spn] = np.where(good, 0.0, -300.0)

    # noob folded into the O->oT copy: denominator rows 48 (e=0) and 112
    # (e=1) get the count of x-out-of-bounds keys (reference zero-pads
    # those -> exp(0) each).
    noobp = np.zeros((128, 512), np.float32)
    for qy in range(16):
        for qx in range(32):
            nb = 7.0 * (max(0, 3 - qx) + max(0, qx - 28))
            noobp[48, 32 * qy + qx] = nb
            noobp[112, 32 * qy + qx] = nb
    sel4 = np.zeros((128, 2, 4), np.float32)
    for j in range(2):
        sel4[48, j, 2 * j] = 1.0
        sel4[112, j, 2 * j + 1] = 1.0
    bsel = np.zeros((4, 2, 128), np.float32)
    for j in range(2):
        bsel[2 * j, j, 0:64] = 1.0
        bsel[2 * j + 1, j, 64:128] = 1.0
    ones1 = np.ones((1, 128), np.float32)
    identm = np.eye(128, dtype=np.float32)
    vtall = np.zeros((128, 6, 8, 16), np.float32)
    vtall[:, :, :, 0] = 1.0
    c = dict(wqk=wqk, wv=wvp, wp=wp, bp=bp, mneg=mneg, noobp=noobp,
             sel4=sel4, ones1=ones1, ident=identm, vtall=vtall)
    c["bsel"] = bsel
    return {k: np.ascontiguousarray(v.astype(BF_NP)) for k, v in c.items()}


def kernel(x, w_qkv, w_proj, b_proj, H=32, W=32):
    global LAST_RESULT
    x = np.asarray(x, np.float32)
    w_qkv = np.asarray(w_qkv, np.float32)
    w_proj = np.asarray(w_proj, np.float32)
    b_proj = np.asarray(b_proj, np.float32)
    assert x.shape == (1, NPOS, C) and int(H) == 32 and int(W) == 32

    nc = _build_nc()
    consts = _host_consts(w_qkv, w_proj, b_proj)

    x4 = x[0].reshape(T, HH, WW, C)
    in_maps = []
    for c in range(8):
        t, ry0 = c // 2, 16 * (c % 2)
        xh = np.zeros((24, WW, C), np.float32)
        lo, hi = ry0 - 3, ry0 + 21
        slo, shi = max(lo, 0), min(hi, HH)
        xh[slo - lo:shi - lo] = x4[t, slo:shi]
        xT = np.ascontiguousarray(
            xh.reshape(768, C).T.reshape(3, 128, 768).transpose(1, 0, 2)
        ).astype(BF_NP)
        in_maps.append({"xT": xT, **consts})

    trace = bool(int(os.environ.get("TRACE", "0")))
    res = run_bass_kernel_spmd(nc, in_maps, core_ids=list(range(8)),
                               trace=trace)
    LAST_RESULT = res
    out = np.concatenate([res.results[c]["out"] for c in range(8)], axis=0)
    return out.reshape(1, NPOS, C)
